# revision 13
# baseline (speedup 1.0000x reference)
"""PointerNet attention scoring kernel for Trainium2 (8 NeuronCores).

Computes, for full inputs:
    q_t = query @ Wq + bq                      # (L_q, B, H)
    h_t = decoder_states @ Wh + bh             # (L_a, B, H)
    s[a,q,b] = sum_h tanh(q_t[q,b,h] + h_t[a,b,h]) * w2[h] (+ b2)
    out[a,b,q] = softmax_q(s[a,q,b])  (mask applied post-exp; ones here)

Sharding: data-parallel over L_a (512 -> 8 x 64). Each core receives the
full (host-pre-transposed) query / weights and its decoder_states slice,
and produces its (64, B, L_q) slice of the output. b2 is dropped
(softmax-invariant); the query mask, if not all ones, is applied
host-side (exactly). Host prep is layout-only (transposes, one-hot
expansion of w2) - all FLOPs stay on device.

Per-core on-chip pipeline (raw Bass, explicit semaphores - the walrus
build here only accepts one embedded sync-wait per instruction, so Tile
is unusable and all cross-engine waits are standalone wait_ge):
  - H=128 lives on partitions. q_tT[h,q] (per b) and per-(a,b) bias
    columns h_tT[h,a]+bq+bh come from small PE matmuls over the
    pre-transposed inputs.
  - Main loop, 16 chunks of CH=16 (a,b) pairs: DVE tensor_scalar_add
    broadcasts a bias column over q to build [128, CH*512] tanh inputs;
    one ScalarE Tanh per chunk (ScalarE is the roofline: 16.8M tanh
    elements / 128 lanes / 1.2 GHz ~= 109us); PE reduces each pair with
    a one-hot-scaled w2 stationary ([128,32], w2 in column i) at
    tile_position (0,32j), accumulating pair (32j+i)'s scores directly
    into PSUM partition 32j+i of a [128,512] scores bank (zero columns
    add exact +0.0).
  - Softmax over q (free axis) per 128-row scores bank: DVE negated
    max, ScalarE Exp with bias=-max and fused accum (row sums), DVE
    reciprocal + scale, DMA out.
"""

import numpy as np

L_Q, L_A, B = 512, 512, 4
Q_SIZE, D_SIZE, H = 256, 512, 128
N_CORES = 8
A_PER = L_A // N_CORES  # 64
CH = 16                 # (a,b) pairs per tanh chunk
NCHUNK = (A_PER * B) // CH          # 16
NTILE = (A_PER * B) // 128          # 2 scores tiles of 128 pair-rows

_CACHE = {}


def build_program():
    from contextlib import ExitStack

    import concourse.bass as bass
    from concourse import mybir

    f32 = mybir.dt.float32
    AF = mybir.ActivationFunctionType
    ALU = mybir.AluOpType
    AX = mybir.AxisListType

    NQC = Q_SIZE // 128  # 2 contraction chunks for q_t
    NDC = D_SIZE // 128  # 4 contraction chunks for h_t
    GPT = 128 // CH      # 8 chunks per scores tile

    nc = bass.Bass()
    qT = nc.declare_dram_parameter("qT", [B, Q_SIZE, L_Q], f32, isOutput=False)
    dT = nc.declare_dram_parameter("dT", [B, D_SIZE, A_PER], f32, isOutput=False)
    wq = nc.declare_dram_parameter("wq", [Q_SIZE, H], f32, isOutput=False)
    wh = nc.declare_dram_parameter("wh", [D_SIZE, H], f32, isOutput=False)
    w2oh_in = nc.declare_dram_parameter("w2oh", [H, 32, 32], f32, isOutput=False)
    bqh = nc.declare_dram_parameter("bqh", [H, 1], f32, isOutput=False)
    out = nc.declare_dram_parameter("out", [A_PER, B, L_Q], f32, isOutput=True)

    with ExitStack() as ctx:
        _n = [0]

        def sb(shape, nm=None):
            _n[0] += 1
            return ctx.enter_context(
                nc.sbuf_tensor(f"sb{_n[0]}", shape, f32)
            )

        def ps(shape, nm=None):
            _n[0] += 1
            return ctx.enter_context(
                nc.psum_tensor(f"ps{_n[0]}", shape, f32)
            )

        wq_sb = sb([128, NQC, H])
        wh_sb = sb([128, NDC, H])
        w2oh = sb([128, 32, 32])
        bqh_sb = sb([128, 1])
        qT_sb = [sb([128, NQC, L_Q]) for _ in range(B)]
        dT_sb = [sb([128, NDC, A_PER]) for _ in range(B)]
        qtt = [sb([128, L_Q]) for _ in range(B)]
        biasc = [sb([128, A_PER]) for _ in range(B)]
        tin = [sb([128, CH * L_Q]) for _ in range(2)]
        probs = [sb([128, L_Q]) for _ in range(NTILE)]
        outt = [sb([128, L_Q]) for _ in range(NTILE)]
        negmax = [sb([128, 1]) for _ in range(NTILE)]
        sumexp = [sb([128, 1]) for _ in range(NTILE)]
        rsum = [sb([128, 1]) for _ in range(NTILE)]

        qt_ps = [ps([128, L_Q]) for _ in range(2)]
        ht_ps = [ps([128, A_PER]) for _ in range(2)]
        scores_ps = [ps([128, L_Q]) for _ in range(2)]

        dsem = ctx.enter_context(nc.semaphore("dsem"))
        psem = ctx.enter_context(nc.semaphore("psem"))
        asem = ctx.enter_context(nc.semaphore("asem"))
        vsem = ctx.enter_context(nc.semaphore("vsem"))
        osem = ctx.enter_context(nc.semaphore("osem"))

        N_IN_DMAS = 4 + 2 * B          # weights + per-b qT/dT
        D_ALL = 16 * N_IN_DMAS         # dsem after all inputs landed

        # --- semaphore milestone precomputation (program order per engine)
        # psem: qt matmuls b0..3 (1..4), ht b0..3 (5..8), then one inc per
        # main chunk.
        p_qt = [b + 1 for b in range(B)]
        p_ht = [B + b + 1 for b in range(B)]
        p_chunk = [2 * B + g + 1 for g in range(NCHUNK)]
        # vsem: qtt copies (1..4), bias adds (5..8), then per chunk one inc
        # for the adds, plus per-tile negmax and output-scale incs woven in.
        v_qtt = [b + 1 for b in range(B)]
        v_bias = [B + b + 1 for b in range(B)]
        v_adds = {}
        v_negmax = {}
        v_out = {}
        v = 2 * B
        for g in range(NCHUNK):
            v += 1
            v_adds[g] = v
            if g == GPT:          # after adds of chunk 8: negmax tile 0
                v += 1
                v_negmax[0] = v
            if g == GPT + 2:      # after adds of chunk 10: scale tile 0
                v += 1
                v_out[0] = v
        v += 1
        v_negmax[1] = v
        v += 1
        v_out[1] = v
        # asem: tanh per chunk, exp0 woven after tanh of chunk 8.
        a_tanh = {}
        a_exp = {}
        a = 0
        for g in range(NCHUNK):
            a += 1
            a_tanh[g] = a
            if g == GPT:
                a += 1
                a_exp[0] = a
        a += 1
        a_exp[1] = a

        with nc.Block() as block:

            @block.sync
            def _(sync):
                sync.dma_start(
                    out=wq_sb[:, :, :],
                    in_=wq[:, :].rearrange("(j p) h -> p j h", p=128),
                ).then_inc(dsem, 16)
                sync.dma_start(
                    out=wh_sb[:, :, :],
                    in_=wh[:, :].rearrange("(j p) h -> p j h", p=128),
                ).then_inc(dsem, 16)
                sync.dma_start(out=w2oh[:, :, :], in_=w2oh_in[:, :, :]).then_inc(
                    dsem, 16
                )
                sync.dma_start(out=bqh_sb[:, :], in_=bqh[:, :]).then_inc(dsem, 16)
                for b in range(B):
                    sync.dma_start(
                        out=qT_sb[b][:, :, :],
                        in_=qT[b, :, :].rearrange("(j p) q -> p j q", p=128),
                    ).then_inc(dsem, 16)
                for b in range(B):
                    sync.dma_start(
                        out=dT_sb[b][:, :, :],
                        in_=dT[b, :, :].rearrange("(j p) a -> p j a", p=128),
                    ).then_inc(dsem, 16)
                # output DMAs, two [64, 512] row-blocks per scores tile
                for t in range(NTILE):
                    sync.wait_ge(vsem, v_out[t])
                    sync.dma_start(
                        out=out[:, 2 * t, :], in_=outt[t][0:A_PER, :]
                    ).then_inc(osem, 16)
                    sync.dma_start(
                        out=out[:, 2 * t + 1, :], in_=outt[t][A_PER:128, :]
                    ).then_inc(osem, 16)
                sync.wait_ge(osem, 16 * 2 * NTILE)

            @block.tensor
            def _(tensor):
                tensor.wait_ge(dsem, D_ALL)
                for b in range(B):
                    if b >= 2:
                        tensor.wait_ge(vsem, v_qtt[b - 2])
                    for j in range(NQC):
                        ins = nc.tensor.matmul(
                            qt_ps[b % 2][:, :],
                            wq_sb[:, j, :],
                            qT_sb[b][:, j, :],
                            start=(j == 0),
                            stop=(j == NQC - 1),
                        )
                    ins.then_inc(psem, 1)
                for b in range(B):
                    if b >= 2:
                        tensor.wait_ge(vsem, v_bias[b - 2])
                    for j in range(NDC):
                        ins = nc.tensor.matmul(
                            ht_ps[b % 2][:, :],
                            wh_sb[:, j, :],
                            dT_sb[b][:, j, :],
                            start=(j == 0),
                            stop=(j == NDC - 1),
                        )
                    ins.then_inc(psem, 1)
                for g in range(NCHUNK):
                    tensor.wait_ge(asem, a_tanh[g])
                    t, gt = divmod(g, GPT)
                    for i in range(CH):
                        r = gt * CH + i
                        j, ii = divmod(r, 32)
                        ins = nc.tensor.matmul(
                            scores_ps[t][32 * j:32 * (j + 1), :],
                            w2oh[:, ii, :],
                            tin[g % 2][:, i * L_Q:(i + 1) * L_Q],
                            start=(ii == 0),
                            stop=(ii == 31),
                            tile_position=(0, 32 * j),
                        )
                    ins.then_inc(psem, 1)

            @block.scalar
            def _(scalar):
                for g in range(NCHUNK):
                    scalar.wait_ge(vsem, v_adds[g])
                    nc.scalar.activation(
                        tin[g % 2][:, :], tin[g % 2][:, :], AF.Tanh
                    ).then_inc(asem, 1)
                    if g == GPT:
                        scalar.wait_ge(psem, p_chunk[GPT - 1])
                        scalar.wait_ge(vsem, v_negmax[0])
                        nc.scalar.activation(
                            probs[0][:, :],
                            scores_ps[0][:, :],
                            AF.Exp,
                            bias=negmax[0][:, :],
                            accum_out=sumexp[0][:, :],
                        ).then_inc(asem, 1)
                scalar.wait_ge(psem, p_chunk[NCHUNK - 1])
                scalar.wait_ge(vsem, v_negmax[1])
                nc.scalar.activation(
                    probs[1][:, :],
                    scores_ps[1][:, :],
                    AF.Exp,
                    bias=negmax[1][:, :],
                    accum_out=sumexp[1][:, :],
                ).then_inc(asem, 1)

            @block.vector
            def _(vector):
                vector.wait_ge(dsem, D_ALL)
                for b in range(B):
                    vector.wait_ge(psem, p_qt[b])
                    nc.vector.tensor_copy(qtt[b][:, :], qt_ps[b % 2][:, :]).then_inc(
                        vsem, 1
                    )
                for b in range(B):
                    vector.wait_ge(psem, p_ht[b])
                    nc.vector.tensor_scalar_add(
                        biasc[b][:, :], ht_ps[b % 2][:, :], bqh_sb[:, :]
                    ).then_inc(vsem, 1)
                for g in range(NCHUNK):
                    if g >= 2:
                        vector.wait_ge(psem, p_chunk[g - 2])
                    b = g // (NCHUNK // B)
                    for i in range(CH):
                        ai = (g % (NCHUNK // B)) * CH + i
                        ins = nc.vector.tensor_scalar_add(
                            tin[g % 2][:, i * L_Q:(i + 1) * L_Q],
                            qtt[b][:, :],
                            biasc[b][:, ai:ai + 1],
                        )
                    ins.then_inc(vsem, 1)
                    if g == GPT:
                        vector.wait_ge(psem, p_chunk[GPT - 1])
                        nc.vector.tensor_reduce(
                            negmax[0][:, :], scores_ps[0][:, :],
                            axis=AX.X, op=ALU.max, negate=True,
                        ).then_inc(vsem, 1)
                    if g == GPT + 2:
                        vector.wait_ge(asem, a_exp[0])
                        nc.vector.reciprocal(rsum[0][:, :], sumexp[0][:, :])
                        vector.drain()
                        nc.vector.tensor_scalar_mul(
                            outt[0][:, :], probs[0][:, :], rsum[0][:, :]
                        ).then_inc(vsem, 1)
                vector.wait_ge(psem, p_chunk[NCHUNK - 1])
                nc.vector.tensor_reduce(
                    negmax[1][:, :], scores_ps[1][:, :],
                    axis=AX.X, op=ALU.max, negate=True,
                ).then_inc(vsem, 1)
                vector.wait_ge(asem, a_exp[1])
                nc.vector.reciprocal(rsum[1][:, :], sumexp[1][:, :])
                vector.drain()
                nc.vector.tensor_scalar_mul(
                    outt[1][:, :], probs[1][:, :], rsum[1][:, :]
                ).then_inc(vsem, 1)

    return nc


def _get_program():
    if "nc" not in _CACHE:
        _CACHE["nc"] = build_program()
    return _CACHE["nc"]


def _make_in_maps(inputs):
    query = np.asarray(inputs["query"], dtype=np.float32)
    decoder_states = np.asarray(inputs["decoder_states"], dtype=np.float32)
    Wq = np.ascontiguousarray(np.asarray(inputs["Wq"], dtype=np.float32))
    Wh = np.ascontiguousarray(np.asarray(inputs["Wh"], dtype=np.float32))
    w2v = np.asarray(inputs["w2"], np.float32).reshape(H)
    w2oh = np.zeros((H, 32, 32), dtype=np.float32)
    w2oh[:, np.arange(32), np.arange(32)] = w2v[:, None]
    bqh = np.ascontiguousarray(
        (np.asarray(inputs["bq"], np.float32)
         + np.asarray(inputs["bh"], np.float32)).reshape(H, 1)
    )
    qT = np.ascontiguousarray(query.transpose(1, 2, 0))  # (B, Q, L_q)
    in_maps = []
    for c in range(N_CORES):
        dslice = decoder_states[c * A_PER:(c + 1) * A_PER]
        in_maps.append({
            "qT": qT,
            "dT": np.ascontiguousarray(dslice.transpose(1, 2, 0)),  # (B, D, A)
            "wq": Wq,
            "wh": Wh,
            "w2oh": w2oh,
            "bqh": bqh,
        })
    return in_maps


def kernel(query, decoder_states, query_mask, Wq, bq, Wh, bh, w2, b2):
    from concourse.bass_utils import run_bass_kernel_spmd

    mask = np.asarray(query_mask)
    nc = _get_program()
    in_maps = _make_in_maps({
        "query": query, "decoder_states": decoder_states,
        "Wq": Wq, "Wh": Wh, "w2": w2, "bq": bq, "bh": bh,
    })
    res = run_bass_kernel_spmd(nc, in_maps, list(range(N_CORES))).results
    out = np.concatenate([res[c]["out"] for c in range(N_CORES)], axis=0)

    if not mask.all():
        # exact post-exp masking + renormalization, host-side
        m = mask.T.astype(np.float32)  # (B, L_q)
        out = out * m[None, :, :]
        out = out / out.sum(axis=-1, keepdims=True)
    return out


# revision 16
# speedup vs baseline: 1.5418x; 1.5418x over previous
"""PointerNet attention scoring kernel for Trainium2 (8 NeuronCores).

Computes, for full inputs:
    q_t = query @ Wq + bq                      # (L_q, B, H)
    h_t = decoder_states @ Wh + bh             # (L_a, B, H)
    s[a,q,b] = sum_h tanh(q_t[q,b,h] + h_t[a,b,h]) * w2[h] (+ b2)
    out[a,b,q] = softmax_q(s[a,q,b])  (mask applied post-exp; ones here)

Sharding: data-parallel over L_a (512 -> 8 x 64). Each core receives the
full (host-pre-transposed) query / weights and its decoder_states slice,
and produces its (64, B, L_q) slice of the output. b2 is dropped
(softmax-invariant); the query mask, if not all ones, is applied
host-side (exactly). Host prep is layout-only - all FLOPs stay on
device.

Per-core on-chip pipeline (raw Bass, explicit semaphores - the walrus
build here only accepts one embedded sync-wait per instruction, so Tile
is unusable and all cross-engine waits are standalone wait_ge):
  - H=128 on partitions. q_tT[h,q] per b and bias columns
    h_tT[h,(b,a)]+bq+bh from small fp32 PE matmuls over pre-transposed
    inputs; results stored bf16 for the main loop.
  - Main loop, 8 chunks of CH=32 (a,b) pairs, all bf16: DVE
    tensor_scalar_add broadcasts a bias column over q ([128, 32*512]
    bf16 -> 4x DVE mode); one in-place ScalarE Tanh per chunk (ScalarE
    is the roofline: 16.8M elems / 128 lanes / 1.2 GHz ~= 109 us); PE
    reduces each pair with a one-hot-scaled bf16 w2 stationary
    ([128,32], w2 in column i) at tile_position (0,32j), accumulating
    pair (32j+i)'s scores into fp32 PSUM partition 32j+i (zero columns
    add exact +0.0; bf16 matvec is single-pass, fp32 would be 2x).
  - Softmax over q (free axis) per 128-row scores bank, fp32: DVE
    negated max, ScalarE Exp with bias=-max and fused row-sum accum,
    DVE reciprocal + scale, DMA out.
"""

import numpy as np

L_Q, L_A, B = 512, 512, 4
Q_SIZE, D_SIZE, H = 256, 512, 128
N_CORES = 8
A_PER = L_A // N_CORES  # 64
CH = 32                 # (a,b) pairs per tanh chunk
NCHUNK = (A_PER * B) // CH          # 8
NTILE = (A_PER * B) // 128          # 2 scores tiles of 128 pair-rows
NAB = A_PER * B                     # 256 pair columns

_CACHE = {}


def build_program():
    from contextlib import ExitStack

    import concourse.bass as bass
    from concourse import mybir

    f32 = mybir.dt.float32
    bf16 = mybir.dt.bfloat16
    AF = mybir.ActivationFunctionType
    ALU = mybir.AluOpType
    AX = mybir.AxisListType

    NQC = Q_SIZE // 128  # 2 contraction chunks for q_t
    NDC = D_SIZE // 128  # 4 contraction chunks for h_t
    GPT = 128 // CH      # 4 chunks per scores tile
    CPB = A_PER // CH    # 2 chunks per batch entry

    nc = bass.Bass()
    qT = nc.declare_dram_parameter("qT", [B, Q_SIZE, L_Q], f32, isOutput=False)
    dT = nc.declare_dram_parameter("dT", [D_SIZE, NAB], f32, isOutput=False)
    wq = nc.declare_dram_parameter("wq", [Q_SIZE, H], f32, isOutput=False)
    wh = nc.declare_dram_parameter("wh", [D_SIZE, H], f32, isOutput=False)
    w2oh_in = nc.declare_dram_parameter("w2oh", [H, 32, 32], bf16, isOutput=False)
    bqh = nc.declare_dram_parameter("bqh", [H, 1], f32, isOutput=False)
    out = nc.declare_dram_parameter("out", [A_PER, B, L_Q], f32, isOutput=True)

    with ExitStack() as ctx:
        _n = [0]

        def sb(shape, dt=f32):
            _n[0] += 1
            return ctx.enter_context(nc.sbuf_tensor(f"sb{_n[0]}", shape, dt))

        def ps(shape):
            _n[0] += 1
            return ctx.enter_context(nc.psum_tensor(f"ps{_n[0]}", shape, f32))

        wq_sb = sb([128, NQC, H])
        wh_sb = sb([128, NDC, H])
        w2oh = sb([128, 32, 32], bf16)
        bqh_sb = sb([128, 1])
        qT_sb = [sb([128, NQC, L_Q]) for _ in range(B)]
        dT_sb = sb([128, NDC, NAB])
        qtt = [sb([128, L_Q], bf16) for _ in range(B)]
        biasc = sb([128, NAB])  # fp32: tensor_scalar scalar1 must be f32
        tin = [sb([128, CH * L_Q], bf16) for _ in range(2)]
        probs = [sb([128, L_Q]) for _ in range(NTILE)]
        outt = [sb([128, L_Q]) for _ in range(NTILE)]
        negmax = [sb([128, 1]) for _ in range(NTILE)]
        sumexp = [sb([128, 1]) for _ in range(NTILE)]
        rsum = [sb([128, 1]) for _ in range(NTILE)]

        qt_ps = [ps([128, L_Q]) for _ in range(2)]
        ht_ps = ps([128, NAB])
        scores_ps = [ps([128, L_Q]) for _ in range(2)]

        dqsem = ctx.enter_context(nc.semaphore("dqsem"))  # wq,wh,qT x4
        ddsem = ctx.enter_context(nc.semaphore("ddsem"))  # bqh,dT,w2oh
        psem = ctx.enter_context(nc.semaphore("psem"))
        asem = ctx.enter_context(nc.semaphore("asem"))
        vsem = ctx.enter_context(nc.semaphore("vsem"))
        osem = ctx.enter_context(nc.semaphore("osem"))

        DQ_ALL = 16 * (2 + B)   # wq, wh, 4x qT
        DD_ALL = 16 * 3         # bqh, dT, w2oh

        # --- semaphore milestones (program order per engine)
        # psem: qt b0..3 (1..4), ht batch (5), then one inc per chunk
        p_qt = [b + 1 for b in range(B)]
        p_ht = B + 1
        p_chunk = [B + 2 + g for g in range(NCHUNK)]
        # vsem: qtt copies (1..4), bias add (5), per chunk adds + woven
        # softmax steps
        v_qtt = [b + 1 for b in range(B)]
        v_bias = B + 1
        v_adds = {}
        v_negmax = {}
        v_out = {}
        v = B + 1
        for g in range(NCHUNK):
            v += 1
            v_adds[g] = v
            if g == GPT:          # after adds of chunk 4: negmax tile 0
                v += 1
                v_negmax[0] = v
            if g == GPT + 1:      # after adds of chunk 5: scale tile 0
                v += 1
                v_out[0] = v
        v += 1
        v_negmax[1] = v
        v += 1
        v_out[1] = v
        # asem: tanh per chunk, exp0 woven after tanh of chunk GPT
        a_tanh = {}
        a_exp = {}
        a = 0
        for g in range(NCHUNK):
            a += 1
            a_tanh[g] = a
            if g == GPT:
                a += 1
                a_exp[0] = a
        a += 1
        a_exp[1] = a

        with nc.Block() as block:

            @block.sync
            def _(sync):
                sync.dma_start(
                    out=wq_sb[:, :, :],
                    in_=wq[:, :].rearrange("(j p) h -> p j h", p=128),
                ).then_inc(dqsem, 16)
                sync.dma_start(
                    out=wh_sb[:, :, :],
                    in_=wh[:, :].rearrange("(j p) h -> p j h", p=128),
                ).then_inc(dqsem, 16)
                for b in range(B):
                    sync.dma_start(
                        out=qT_sb[b][:, :, :],
                        in_=qT[b, :, :].rearrange("(j p) q -> p j q", p=128),
                    ).then_inc(dqsem, 16)
                sync.dma_start(out=bqh_sb[:, :], in_=bqh[:, :]).then_inc(ddsem, 16)
                sync.dma_start(
                    out=dT_sb[:, :, :],
                    in_=dT[:, :].rearrange("(j p) a -> p j a", p=128),
                ).then_inc(ddsem, 16)
                sync.dma_start(out=w2oh[:, :, :], in_=w2oh_in[:, :, :]).then_inc(
                    ddsem, 16
                )
                # output DMAs, two [64, 512] row-blocks per scores tile
                for t in range(NTILE):
                    sync.wait_ge(vsem, v_out[t])
                    sync.dma_start(
                        out=out[:, 2 * t, :], in_=outt[t][0:A_PER, :]
                    ).then_inc(osem, 16)
                    sync.dma_start(
                        out=out[:, 2 * t + 1, :], in_=outt[t][A_PER:128, :]
                    ).then_inc(osem, 16)
                sync.wait_ge(osem, 16 * 2 * NTILE)

            @block.tensor
            def _(tensor):
                tensor.wait_ge(dqsem, DQ_ALL)
                for b in range(B):
                    if b >= 2:
                        tensor.wait_ge(vsem, v_qtt[b - 2])
                    for j in range(NQC):
                        ins = nc.tensor.matmul(
                            qt_ps[b % 2][:, :],
                            wq_sb[:, j, :],
                            qT_sb[b][:, j, :],
                            start=(j == 0),
                            stop=(j == NQC - 1),
                        )
                    ins.then_inc(psem, 1)
                tensor.wait_ge(ddsem, DD_ALL)
                for j in range(NDC):
                    ins = nc.tensor.matmul(
                        ht_ps[:, :],
                        wh_sb[:, j, :],
                        dT_sb[:, j, :],
                        start=(j == 0),
                        stop=(j == NDC - 1),
                    )
                ins.then_inc(psem, 1)
                for g in range(NCHUNK):
                    tensor.wait_ge(asem, a_tanh[g])
                    t, gt = divmod(g, GPT)
                    for i in range(CH):
                        ins = nc.tensor.matmul(
                            scores_ps[t][32 * gt:32 * (gt + 1), :],
                            w2oh[:, i, :],
                            tin[g % 2][:, i * L_Q:(i + 1) * L_Q],
                            start=(i == 0),
                            stop=(i == CH - 1),
                            tile_position=(0, 32 * gt),
                        )
                    ins.then_inc(psem, 1)

            @block.scalar
            def _(scalar):
                for g in range(NCHUNK):
                    scalar.wait_ge(vsem, v_adds[g])
                    nc.scalar.activation(
                        tin[g % 2][:, :], tin[g % 2][:, :], AF.Tanh
                    ).then_inc(asem, 1)
                    if g == GPT:
                        scalar.wait_ge(psem, p_chunk[GPT - 1])
                        scalar.wait_ge(vsem, v_negmax[0])
                        nc.scalar.activation(
                            probs[0][:, :],
                            scores_ps[0][:, :],
                            AF.Exp,
                            bias=negmax[0][:, :],
                            accum_out=sumexp[0][:, :],
                        ).then_inc(asem, 1)
                scalar.wait_ge(psem, p_chunk[NCHUNK - 1])
                scalar.wait_ge(vsem, v_negmax[1])
                nc.scalar.activation(
                    probs[1][:, :],
                    scores_ps[1][:, :],
                    AF.Exp,
                    bias=negmax[1][:, :],
                    accum_out=sumexp[1][:, :],
                ).then_inc(asem, 1)

            @block.vector
            def _(vector):
                for b in range(B):
                    vector.wait_ge(psem, p_qt[b])
                    nc.vector.tensor_copy(qtt[b][:, :], qt_ps[b % 2][:, :]).then_inc(
                        vsem, 1
                    )
                vector.wait_ge(psem, p_ht)
                nc.vector.tensor_scalar_add(
                    biasc[:, :], ht_ps[:, :], bqh_sb[:, :]
                ).then_inc(vsem, 1)
                vector.drain()
                for g in range(NCHUNK):
                    if g >= 2:
                        vector.wait_ge(psem, p_chunk[g - 2])
                    b = g // CPB
                    for i in range(CH):
                        ci = b * A_PER + (g % CPB) * CH + i
                        ins = nc.vector.tensor_scalar_add(
                            tin[g % 2][:, i * L_Q:(i + 1) * L_Q],
                            qtt[b][:, :],
                            biasc[:, ci:ci + 1],
                        )
                    ins.then_inc(vsem, 1)
                    if g == GPT:
                        vector.wait_ge(psem, p_chunk[GPT - 1])
                        nc.vector.tensor_reduce(
                            negmax[0][:, :], scores_ps[0][:, :],
                            axis=AX.X, op=ALU.max, negate=True,
                        ).then_inc(vsem, 1)
                    if g == GPT + 1:
                        vector.wait_ge(asem, a_exp[0])
                        nc.vector.reciprocal(rsum[0][:, :], sumexp[0][:, :])
                        vector.drain()
                        nc.vector.tensor_scalar_mul(
                            outt[0][:, :], probs[0][:, :], rsum[0][:, :]
                        ).then_inc(vsem, 1)
                vector.wait_ge(psem, p_chunk[NCHUNK - 1])
                nc.vector.tensor_reduce(
                    negmax[1][:, :], scores_ps[1][:, :],
                    axis=AX.X, op=ALU.max, negate=True,
                ).then_inc(vsem, 1)
                vector.wait_ge(asem, a_exp[1])
                nc.vector.reciprocal(rsum[1][:, :], sumexp[1][:, :])
                vector.drain()
                nc.vector.tensor_scalar_mul(
                    outt[1][:, :], probs[1][:, :], rsum[1][:, :]
                ).then_inc(vsem, 1)

    return nc


def _get_program():
    if "nc" not in _CACHE:
        _CACHE["nc"] = build_program()
    return _CACHE["nc"]


def _make_in_maps(inputs):
    import ml_dtypes

    query = np.asarray(inputs["query"], dtype=np.float32)
    decoder_states = np.asarray(inputs["decoder_states"], dtype=np.float32)
    Wq = np.ascontiguousarray(np.asarray(inputs["Wq"], dtype=np.float32))
    Wh = np.ascontiguousarray(np.asarray(inputs["Wh"], dtype=np.float32))
    w2v = np.asarray(inputs["w2"], np.float32).reshape(H)
    w2oh = np.zeros((H, 32, 32), dtype=np.float32)
    w2oh[:, np.arange(32), np.arange(32)] = w2v[:, None]
    w2oh = w2oh.astype(ml_dtypes.bfloat16)
    bqh = np.ascontiguousarray(
        (np.asarray(inputs["bq"], np.float32)
         + np.asarray(inputs["bh"], np.float32)).reshape(H, 1)
    )
    qT = np.ascontiguousarray(query.transpose(1, 2, 0))  # (B, Q, L_q)
    in_maps = []
    for c in range(N_CORES):
        dslice = decoder_states[c * A_PER:(c + 1) * A_PER]
        # (D, B*A): column (b*A + a) holds decoder_states[a, b, :]
        dT = np.ascontiguousarray(
            dslice.transpose(2, 1, 0).reshape(D_SIZE, NAB)
        )
        in_maps.append({
            "qT": qT,
            "dT": dT,
            "wq": Wq,
            "wh": Wh,
            "w2oh": w2oh,
            "bqh": bqh,
        })
    return in_maps


def kernel(query, decoder_states, query_mask, Wq, bq, Wh, bh, w2, b2):
    from concourse.bass_utils import run_bass_kernel_spmd

    mask = np.asarray(query_mask)
    nc = _get_program()
    in_maps = _make_in_maps({
        "query": query, "decoder_states": decoder_states,
        "Wq": Wq, "Wh": Wh, "w2": w2, "bq": bq, "bh": bh,
    })
    res = run_bass_kernel_spmd(nc, in_maps, list(range(N_CORES))).results
    out = np.concatenate([res[c]["out"] for c in range(N_CORES)], axis=0)

    if not mask.all():
        # exact post-exp masking + renormalization, host-side
        m = mask.T.astype(np.float32)  # (B, L_q)
        out = out * m[None, :, :]
        out = out / out.sum(axis=-1, keepdims=True)
    return out


# revision 17
# speedup vs baseline: 1.7611x; 1.1422x over previous
"""PointerNet attention scoring kernel for Trainium2 (8 NeuronCores).

Computes, for full inputs:
    q_t = query @ Wq + bq                      # (L_q, B, H)
    h_t = decoder_states @ Wh + bh             # (L_a, B, H)
    s[a,q,b] = sum_h tanh(q_t[q,b,h] + h_t[a,b,h]) * w2[h] (+ b2)
    out[a,b,q] = softmax_q(s[a,q,b])  (mask applied post-exp; ones here)

Sharding: data-parallel over L_a (512 -> 8 x 64). Each core receives the
full (host-pre-transposed) query / weights and its decoder_states slice,
and produces a row-permuted (256, 512) block that the host scatters into
the (64, B, L_q) output slice. b2 is dropped (softmax-invariant); the
query mask, if not all ones, is applied host-side (exactly). Host prep
is layout-only - all FLOPs stay on device.

Per-core on-chip pipeline (raw Bass, explicit semaphores - the walrus
build here only accepts one embedded sync-wait per instruction, so Tile
is unusable and all cross-engine waits are standalone wait_ge):
  - H=128 on partitions. q_tT[h,q] per b and bias columns
    h_tT[h,(b,a)]+bq+bh from small fp32 PE matmuls over pre-transposed
    inputs; stored bf16/f32 for the main loop.
  - Main loop, 8 chunks of CH=32 (a,b) pairs (first/last chunk split in
    two for pipeline ramp), bf16 datapath: DVE tensor_scalar_add
    broadcasts a bias column over q; one in-place ScalarE Tanh per
    chunk-part (ScalarE is the roofline: 16.8M elems / 128 lanes /
    1.2 GHz ~= 109 us); PE reduces each pair with a one-hot-scaled bf16
    w2 stationary ([128,32], w2 in column v) at tile_position (0,32j).
    Consecutive matvecs rotate over the 4 column-groups (4 separate
    PSUM banks) so they run concurrently in the PE array; pair k of
    chunk gt lands in bank k%4, partition 32*(k%4) + 8*gt + k//4 (the
    31 zero stationary columns accumulate exact +0.0).
  - Softmax over q (free axis), fp32, in 4 bank-pieces of 32 rows: DVE
    negated max, ScalarE Exp with bias=-max and fused row-sum accum,
    DVE reciprocal + scale, one 256 KB DMA per 128-row tile.
"""

import numpy as np

L_Q, L_A, B = 512, 512, 4
Q_SIZE, D_SIZE, H = 256, 512, 128
N_CORES = 8
A_PER = L_A // N_CORES  # 64
CH = 32                 # (a,b) pairs per tanh chunk
NCHUNK = (A_PER * B) // CH          # 8
NTILE = (A_PER * B) // 128          # 2 scores tiles of 128 pair-rows
NAB = A_PER * B                     # 256 pair rows
GPT = 128 // CH                     # 4 chunks per scores tile
CPB = A_PER // CH                   # 2 chunks per batch entry

_CACHE = {}


def _parts_of(g):
    return 2 if g in (0, NCHUNK - 1) else 1


def _part_ks(g, pt):
    n = _parts_of(g)
    lo = pt * (CH // n)
    return range(lo, lo + CH // n)


def _row_perm():
    """perm[a, b] = raw row index holding out[a, b, :]."""
    perm = np.empty((A_PER, B), dtype=np.int64)
    for g in range(NCHUNK):
        t, gt = divmod(g, GPT)
        b = g // CPB
        for k in range(CH):
            a = (g % CPB) * CH + k
            perm[a, b] = t * 128 + 32 * (k % 4) + 8 * gt + k // 4
    return perm


def build_program():
    from contextlib import ExitStack

    import concourse.bass as bass
    from concourse import mybir

    f32 = mybir.dt.float32
    bf16 = mybir.dt.bfloat16
    AF = mybir.ActivationFunctionType
    ALU = mybir.AluOpType
    AX = mybir.AxisListType

    NQC = Q_SIZE // 128   # 2 contraction chunks for q_t
    NDC = D_SIZE // 128   # 4 contraction chunks for h_t
    NWC = NQC + NDC       # combined wq|wh chunks

    nc = bass.Bass()
    qT = nc.declare_dram_parameter("qT", [B, Q_SIZE, L_Q], f32, isOutput=False)
    dT = nc.declare_dram_parameter("dT", [D_SIZE, NAB], f32, isOutput=False)
    wqh = nc.declare_dram_parameter("wqh", [Q_SIZE + D_SIZE, H], f32, isOutput=False)
    w2oh_in = nc.declare_dram_parameter("w2oh", [H, 32, 32], bf16, isOutput=False)
    bqh = nc.declare_dram_parameter("bqh", [H, 1], f32, isOutput=False)
    raw = nc.declare_dram_parameter("raw", [NAB, L_Q], f32, isOutput=True)

    with ExitStack() as ctx:
        _n = [0]

        def sb(shape, dt=f32):
            _n[0] += 1
            return ctx.enter_context(nc.sbuf_tensor(f"sb{_n[0]}", shape, dt))

        def ps(shape):
            _n[0] += 1
            return ctx.enter_context(nc.psum_tensor(f"ps{_n[0]}", shape, f32))

        wqh_sb = sb([128, NWC, H])
        w2oh = sb([128, 32, 32], bf16)
        bqh_sb = sb([128, 1])
        qT_sb = [sb([128, NQC, L_Q]) for _ in range(B)]
        dT_sb = sb([128, NDC, NAB])
        qtt = [sb([128, L_Q], bf16) for _ in range(B)]
        biasc = sb([128, NAB])  # fp32: tensor_scalar scalar1 must be f32
        tin = [sb([128, CH * L_Q], bf16) for _ in range(2)]
        probs = [sb([128, L_Q]) for _ in range(NTILE)]
        outt = [sb([128, L_Q]) for _ in range(NTILE)]
        negmax = [sb([128, 1]) for _ in range(NTILE)]
        sumexp = [sb([128, 1]) for _ in range(NTILE)]
        rsum = [sb([128, 1]) for _ in range(NTILE)]

        qt_ps = [ps([128, L_Q]) for _ in range(2)]
        ht_ps = ps([128, NAB])
        banks = [ps([128, L_Q]) for _ in range(4)]  # per col-group scores

        wsem = ctx.enter_context(nc.semaphore("wsem"))
        qsem = [
            ctx.enter_context(nc.semaphore(f"qsem{b}")) for b in range(B)
        ]
        dtsem = ctx.enter_context(nc.semaphore("dtsem"))
        bqsem = ctx.enter_context(nc.semaphore("bqsem"))
        w2sem = ctx.enter_context(nc.semaphore("w2sem"))
        psem = ctx.enter_context(nc.semaphore("psem"))
        asem = ctx.enter_context(nc.semaphore("asem"))
        vsem = ctx.enter_context(nc.semaphore("vsem"))
        osem = ctx.enter_context(nc.semaphore("osem"))

        # --- semaphore milestones (mirror each engine's program order)
        pc = 0
        p_qt = []
        for b in range(B):
            pc += 1
            p_qt.append(pc)
        pc += 1
        p_ht = pc
        p_chunk = {}
        for g in range(NCHUNK):
            for pt in range(_parts_of(g)):
                pc += 1
                p_chunk[(g, pt)] = pc

        def p_last(g):
            return p_chunk[(g, _parts_of(g) - 1)]

        ac = 0
        a_tanh = {}
        a_exp = {}
        for g in range(NCHUNK):
            for pt in range(_parts_of(g)):
                ac += 1
                a_tanh[(g, pt)] = ac
            if g == GPT:
                ac += 1
                a_exp[0] = ac
        ac += 1
        a_exp[1] = ac

        vc = 0
        v_qtt = []
        for b in range(B):
            vc += 1
            v_qtt.append(vc)
        vc += 1
        v_bias = vc
        v_adds = {}
        v_negmax = {}
        v_out = {}
        for g in range(NCHUNK):
            for pt in range(_parts_of(g)):
                vc += 1
                v_adds[(g, pt)] = vc
            if g == GPT:
                vc += 1
                v_negmax[0] = vc
            if g == GPT + 1:
                vc += 1
                v_out[0] = vc
        vc += 1
        v_negmax[1] = vc
        vc += 1
        v_out[1] = vc

        with nc.Block() as block:

            @block.sync
            def _(sync):
                sync.dma_start(
                    out=qT_sb[0][:, :, :],
                    in_=qT[0, :, :].rearrange("(j p) q -> p j q", p=128),
                ).then_inc(qsem[0], 16)
                sync.dma_start(
                    out=wqh_sb[:, :, :],
                    in_=wqh[:, :].rearrange("(j p) h -> p j h", p=128),
                ).then_inc(wsem, 16)
                for b in range(1, B):
                    sync.dma_start(
                        out=qT_sb[b][:, :, :],
                        in_=qT[b, :, :].rearrange("(j p) q -> p j q", p=128),
                    ).then_inc(qsem[b], 16)
                sync.dma_start(
                    out=dT_sb[:, :, :],
                    in_=dT[:, :].rearrange("(j p) a -> p j a", p=128),
                ).then_inc(dtsem, 16)
                sync.dma_start(out=bqh_sb[:, :], in_=bqh[:, :]).then_inc(bqsem, 16)
                sync.dma_start(out=w2oh[:, :, :], in_=w2oh_in[:, :, :]).then_inc(
                    w2sem, 16
                )
                for t in range(NTILE):
                    sync.wait_ge(vsem, v_out[t])
                    sync.dma_start(
                        out=raw[t * 128:(t + 1) * 128, :], in_=outt[t][:, :]
                    ).then_inc(osem, 16)
                sync.wait_ge(osem, 16 * NTILE)

            @block.tensor
            def _(tensor):
                tensor.wait_ge(wsem, 16)
                for b in range(B):
                    tensor.wait_ge(qsem[b], 16)
                    if b >= 2:
                        tensor.wait_ge(vsem, v_qtt[b - 2])
                    for j in range(NQC):
                        ins = nc.tensor.matmul(
                            qt_ps[b % 2][:, :],
                            wqh_sb[:, j, :],
                            qT_sb[b][:, j, :],
                            start=(j == 0),
                            stop=(j == NQC - 1),
                        )
                    ins.then_inc(psem, 1)
                tensor.wait_ge(dtsem, 16)
                for j in range(NDC):
                    ins = nc.tensor.matmul(
                        ht_ps[:, :],
                        wqh_sb[:, NQC + j, :],
                        dT_sb[:, j, :],
                        start=(j == 0),
                        stop=(j == NDC - 1),
                    )
                ins.then_inc(psem, 1)
                tensor.wait_ge(w2sem, 16)
                for g in range(NCHUNK):
                    t, gt = divmod(g, GPT)
                    if t == 1 and gt == 0:
                        # tile 0's banks must be fully read before reuse
                        tensor.wait_ge(vsem, v_negmax[0])
                        tensor.wait_ge(asem, a_exp[0])
                    for pt in range(_parts_of(g)):
                        tensor.wait_ge(asem, a_tanh[(g, pt)])
                        for k in _part_ks(g, pt):
                            j = k % 4
                            v = 8 * gt + k // 4
                            ins = nc.tensor.matmul(
                                banks[j][32 * j:32 * (j + 1), :],
                                w2oh[:, v, :],
                                tin[g % 2][:, k * L_Q:(k + 1) * L_Q],
                                start=(gt == 0 and k < 4),
                                stop=(gt == GPT - 1 and k >= CH - 4),
                                tile_position=(0, 32 * j),
                            )
                        ins.then_inc(psem, 1)

            @block.scalar
            def _(scalar):
                def exp_tile(t):
                    for j in range(4):
                        ins = nc.scalar.activation(
                            probs[t][32 * j:32 * (j + 1), :],
                            banks[j][32 * j:32 * (j + 1), :],
                            AF.Exp,
                            bias=negmax[t][32 * j:32 * (j + 1), :],
                            accum_out=sumexp[t][32 * j:32 * (j + 1), :],
                        )
                    ins.then_inc(asem, 1)

                for g in range(NCHUNK):
                    for pt in range(_parts_of(g)):
                        scalar.wait_ge(vsem, v_adds[(g, pt)])
                        n = _parts_of(g)
                        w = (CH // n) * L_Q
                        nc.scalar.activation(
                            tin[g % 2][:, pt * w:(pt + 1) * w],
                            tin[g % 2][:, pt * w:(pt + 1) * w],
                            AF.Tanh,
                        ).then_inc(asem, 1)
                    if g == GPT:
                        scalar.wait_ge(psem, p_last(GPT - 1))
                        scalar.wait_ge(vsem, v_negmax[0])
                        exp_tile(0)
                scalar.wait_ge(psem, p_last(NCHUNK - 1))
                scalar.wait_ge(vsem, v_negmax[1])
                exp_tile(1)

            @block.vector
            def _(vector):
                def negmax_tile(t):
                    for j in range(4):
                        ins = nc.vector.tensor_reduce(
                            negmax[t][32 * j:32 * (j + 1), :],
                            banks[j][32 * j:32 * (j + 1), :],
                            axis=AX.X, op=ALU.max, negate=True,
                        )
                    ins.then_inc(vsem, 1)

                def scale_tile(t):
                    nc.vector.reciprocal(rsum[t][:, :], sumexp[t][:, :])
                    vector.drain()
                    nc.vector.tensor_scalar_mul(
                        outt[t][:, :], probs[t][:, :], rsum[t][:, :]
                    ).then_inc(vsem, 1)

                for b in range(B):
                    vector.wait_ge(psem, p_qt[b])
                    nc.vector.tensor_copy(qtt[b][:, :], qt_ps[b % 2][:, :]).then_inc(
                        vsem, 1
                    )
                vector.wait_ge(psem, p_ht)
                vector.wait_ge(bqsem, 16)
                nc.vector.tensor_scalar_add(
                    biasc[:, :], ht_ps[:, :], bqh_sb[:, :]
                ).then_inc(vsem, 1)
                vector.drain()
                for g in range(NCHUNK):
                    b = g // CPB
                    for pt in range(_parts_of(g)):
                        if g >= 2:
                            vector.wait_ge(psem, p_last(g - 2))
                        for k in _part_ks(g, pt):
                            ci = b * A_PER + (g % CPB) * CH + k
                            ins = nc.vector.tensor_scalar_add(
                                tin[g % 2][:, k * L_Q:(k + 1) * L_Q],
                                qtt[b][:, :],
                                biasc[:, ci:ci + 1],
                            )
                        ins.then_inc(vsem, 1)
                    if g == GPT:
                        vector.wait_ge(psem, p_last(GPT - 1))
                        negmax_tile(0)
                    if g == GPT + 1:
                        vector.wait_ge(asem, a_exp[0])
                        scale_tile(0)
                vector.wait_ge(psem, p_last(NCHUNK - 1))
                negmax_tile(1)
                vector.wait_ge(asem, a_exp[1])
                scale_tile(1)

    return nc


def _get_program():
    if "nc" not in _CACHE:
        _CACHE["nc"] = build_program()
    return _CACHE["nc"]


def _make_in_maps(inputs):
    import ml_dtypes

    query = np.asarray(inputs["query"], dtype=np.float32)
    decoder_states = np.asarray(inputs["decoder_states"], dtype=np.float32)
    Wq = np.asarray(inputs["Wq"], dtype=np.float32)
    Wh = np.asarray(inputs["Wh"], dtype=np.float32)
    wqh = np.ascontiguousarray(np.vstack([Wq, Wh]))
    w2v = np.asarray(inputs["w2"], np.float32).reshape(H)
    w2oh = np.zeros((H, 32, 32), dtype=np.float32)
    w2oh[:, np.arange(32), np.arange(32)] = w2v[:, None]
    w2oh = w2oh.astype(ml_dtypes.bfloat16)
    bqh = np.ascontiguousarray(
        (np.asarray(inputs["bq"], np.float32)
         + np.asarray(inputs["bh"], np.float32)).reshape(H, 1)
    )
    qT = np.ascontiguousarray(query.transpose(1, 2, 0))  # (B, Q, L_q)
    in_maps = []
    for c in range(N_CORES):
        dslice = decoder_states[c * A_PER:(c + 1) * A_PER]
        # (D, B*A): column (b*A + a) holds decoder_states[a, b, :]
        dT = np.ascontiguousarray(
            dslice.transpose(2, 1, 0).reshape(D_SIZE, NAB)
        )
        in_maps.append({
            "qT": qT,
            "dT": dT,
            "wqh": wqh,
            "w2oh": w2oh,
            "bqh": bqh,
        })
    return in_maps


def kernel(query, decoder_states, query_mask, Wq, bq, Wh, bh, w2, b2):
    from concourse.bass_utils import run_bass_kernel_spmd

    mask = np.asarray(query_mask)
    nc = _get_program()
    in_maps = _make_in_maps({
        "query": query, "decoder_states": decoder_states,
        "Wq": Wq, "Wh": Wh, "w2": w2, "bq": bq, "bh": bh,
    })
    res = run_bass_kernel_spmd(nc, in_maps, list(range(N_CORES))).results
    perm = _row_perm()  # (A_PER, B) -> raw row
    out = np.empty((L_A, B, L_Q), dtype=np.float32)
    for c in range(N_CORES):
        out[c * A_PER:(c + 1) * A_PER] = res[c]["raw"][perm, :]

    if not mask.all():
        # exact post-exp masking + renormalization, host-side
        m = mask.T.astype(np.float32)  # (B, L_q)
        out = out * m[None, :, :]
        out = out / out.sum(axis=-1, keepdims=True)
    return out


# revision 18
# speedup vs baseline: 1.7835x; 1.0128x over previous
"""PointerNet attention scoring kernel for Trainium2 (8 NeuronCores).

Computes, for full inputs:
    q_t = query @ Wq + bq                      # (L_q, B, H)
    h_t = decoder_states @ Wh + bh             # (L_a, B, H)
    s[a,q,b] = sum_h tanh(q_t[q,b,h] + h_t[a,b,h]) * w2[h] (+ b2)
    out[a,b,q] = softmax_q(s[a,q,b])  (mask applied post-exp; ones here)

Sharding: data-parallel over L_a (512 -> 8 x 64). Each core receives the
full (host-pre-arranged, partition-major) query / weights and its
decoder_states slice, and produces a row-permuted (256, 512) block that
the host scatters into the (64, B, L_q) output slice. b2 is dropped
(softmax-invariant); the query mask, if not all ones, is applied
host-side (exactly). Host prep is layout-only - all FLOPs stay on
device.

Per-core on-chip pipeline (raw Bass, explicit semaphores - the walrus
build here only accepts one embedded sync-wait per instruction, so Tile
is unusable and all cross-engine waits are standalone wait_ge):
  - H=128 on partitions. q_tT[h,q] per b and bias columns
    h_tT[h,(b,a)]+bq+bh from small fp32 PE matmuls; stored bf16/f32.
  - Main loop, 8 chunks of CH=32 (a,b) pairs (first/last chunk split in
    two for pipeline ramp), bf16 datapath: DVE tensor_scalar_add
    broadcasts a bias column over q; one in-place ScalarE Tanh per
    chunk-part (ScalarE is the roofline: 16.8M elems / 128 lanes /
    1.2 GHz ~= 109 us); PE reduces each pair with a one-hot-scaled bf16
    w2 stationary ([128,32], w2 in column v) at tile_position (0,32j),
    accumulating into PSUM partition 32j+v of per-column-group banks
    (the 31 zero stationary columns add exact +0.0; bf16 matvecs are
    single-pass where fp32 would be two).
  - Scores tile 0 interleaves its matvecs over all 4 column-groups
    (4-way PE concurrency); its softmax hides under the next tanh.
    Tile 1 fills groups {0,1} during chunks 4-5 and {2,3} during 6-7
    (2-way concurrency) so half its softmax also hides under tanh and
    only groups 2,3 drain at the kernel tail.
  - Softmax over q (free axis), fp32, per 32-row bank piece: DVE
    negated max, ScalarE Exp with bias=-max and fused row-sum accum,
    DVE reciprocal + scale, 128/256 KB output DMAs.
"""

import numpy as np

L_Q, L_A, B = 512, 512, 4
Q_SIZE, D_SIZE, H = 256, 512, 128
N_CORES = 8
A_PER = L_A // N_CORES  # 64
CH = 32                 # (a,b) pairs per tanh chunk
NCHUNK = (A_PER * B) // CH          # 8
NTILE = (A_PER * B) // 128          # 2 scores tiles of 128 pair-rows
NAB = A_PER * B                     # 256 pair rows
GPT = 128 // CH                     # 4 chunks per scores tile
CPB = A_PER // CH                   # 2 chunks per batch entry
NQC = Q_SIZE // 128                 # 2 contraction chunks for q_t
NDC = D_SIZE // 128                 # 4 contraction chunks for h_t
NWC = NQC + NDC

_CACHE = {}


def _parts_of(g):
    return 2 if g in (0, NCHUNK - 1) else 1


def _part_ks(g, pt):
    n = _parts_of(g)
    lo = pt * (CH // n)
    return range(lo, lo + CH // n)


def _mm_plan(g, k):
    """(bank j, one-hot column v, start, stop) for pair-block k of chunk g."""
    t, gt = divmod(g, GPT)
    if t == 0:
        j = k % 4
        v = 8 * gt + k // 4
        return j, v, (gt == 0 and k < 4), (gt == GPT - 1 and k >= CH - 4)
    j = 2 * (gt // 2) + k % 2
    v = 16 * (gt % 2) + k // 2
    return j, v, (gt % 2 == 0 and k < 2), (gt % 2 == 1 and k >= CH - 2)


def _row_perm():
    """perm[a, b] = raw row index holding out[a, b, :]."""
    perm = np.empty((A_PER, B), dtype=np.int64)
    for g in range(NCHUNK):
        t = g // GPT
        b = g // CPB
        for k in range(CH):
            a = (g % CPB) * CH + k
            j, v, _, _ = _mm_plan(g, k)
            perm[a, b] = t * 128 + 32 * j + v
    return perm


def build_program():
    from contextlib import ExitStack

    import concourse.bass as bass
    from concourse import mybir

    f32 = mybir.dt.float32
    bf16 = mybir.dt.bfloat16
    AF = mybir.ActivationFunctionType
    ALU = mybir.AluOpType
    AX = mybir.AxisListType

    nc = bass.Bass()
    qT = nc.declare_dram_parameter("qT", [B, 128, NQC, L_Q], f32, isOutput=False)
    dT = nc.declare_dram_parameter("dT", [128, NDC, NAB], f32, isOutput=False)
    wqh = nc.declare_dram_parameter("wqh", [128, NWC, H], f32, isOutput=False)
    w2oh_in = nc.declare_dram_parameter("w2oh", [H, 32, 32], bf16, isOutput=False)
    bqh = nc.declare_dram_parameter("bqh", [H, 1], f32, isOutput=False)
    raw = nc.declare_dram_parameter("raw", [NAB, L_Q], f32, isOutput=True)

    with ExitStack() as ctx:
        _n = [0]

        def sb(shape, dt=f32):
            _n[0] += 1
            return ctx.enter_context(nc.sbuf_tensor(f"sb{_n[0]}", shape, dt))

        def ps(shape):
            _n[0] += 1
            return ctx.enter_context(nc.psum_tensor(f"ps{_n[0]}", shape, f32))

        wqh_sb = sb([128, NWC, H])
        w2oh = sb([128, 32, 32], bf16)
        bqh_sb = sb([128, 1])
        qT_sb = [sb([128, NQC, L_Q]) for _ in range(B)]
        dT_sb = sb([128, NDC, NAB])
        qtt = [sb([128, L_Q], bf16) for _ in range(B)]
        biasc = sb([128, NAB])  # fp32: tensor_scalar scalar1 must be f32
        tin = [sb([128, CH * L_Q], bf16) for _ in range(2)]
        probs = [sb([128, L_Q]) for _ in range(NTILE)]
        outt = [sb([128, L_Q]) for _ in range(NTILE)]
        negmax = [sb([128, 1]) for _ in range(NTILE)]
        sumexp = [sb([128, 1]) for _ in range(NTILE)]
        rsum = [sb([128, 1]) for _ in range(NTILE)]

        qt_ps = [ps([128, L_Q]) for _ in range(2)]
        ht_ps = ps([128, NAB])
        banks = [ps([128, L_Q]) for _ in range(4)]  # per col-group scores

        wsem = ctx.enter_context(nc.semaphore("wsem"))
        qsem = [ctx.enter_context(nc.semaphore(f"qsem{b}")) for b in range(B)]
        dtsem = ctx.enter_context(nc.semaphore("dtsem"))
        bqsem = ctx.enter_context(nc.semaphore("bqsem"))
        w2sem = ctx.enter_context(nc.semaphore("w2sem"))
        psem = ctx.enter_context(nc.semaphore("psem"))
        asem = ctx.enter_context(nc.semaphore("asem"))
        vsem = ctx.enter_context(nc.semaphore("vsem"))
        osem = ctx.enter_context(nc.semaphore("osem"))

        # --- semaphore milestones (mirror each engine's program order)
        # psem: qt b0 (1), ht (2), qt b1..b3 (3..5), then per chunk-part
        pc = 0
        pc += 1
        p_qt = {0: pc}
        pc += 1
        p_ht = pc
        for b in range(1, B):
            pc += 1
            p_qt[b] = pc
        p_chunk = {}
        for g in range(NCHUNK):
            for pt in range(_parts_of(g)):
                pc += 1
                p_chunk[(g, pt)] = pc

        def p_last(g):
            return p_chunk[(g, _parts_of(g) - 1)]

        # asem: tanh per chunk-part; exp0 (4 pieces) after tanh(4);
        # exp1 pieces {0,1} after tanh(7,0); pieces {2,3} at the end
        ac = 0
        a_tanh = {}
        for g in range(NCHUNK):
            for pt in range(_parts_of(g)):
                ac += 1
                a_tanh[(g, pt)] = ac
                if (g, pt) == (GPT, 0):
                    ac += 1
                    a_exp0 = ac
                if (g, pt) == (NCHUNK - 1, 1):
                    ac += 1
                    a_exp1a = ac
        ac += 1
        a_exp1b = ac

        # vsem: qtt0 (1), bias (2), qtt1..3 (3..5), per chunk-part adds,
        # plus woven softmax steps
        vc = 0
        vc += 1
        v_qtt = {0: vc}
        vc += 1
        v_bias = vc
        for b in range(1, B):
            vc += 1
            v_qtt[b] = vc
        v_adds = {}
        for g in range(NCHUNK):
            for pt in range(_parts_of(g)):
                vc += 1
                v_adds[(g, pt)] = vc
            if g == GPT:
                vc += 1
                v_negmax0 = vc
            if g == GPT + 1:
                vc += 1
                v_out0 = vc
            if g == NCHUNK - 2:
                vc += 1
                v_negmax1a = vc
        vc += 1
        v_negmax1b = vc
        vc += 1
        v_out1a = vc
        vc += 1
        v_out1b = vc

        with nc.Block() as block:

            @block.sync
            def _(sync):
                sync.dma_start(out=qT_sb[0][:, :, :], in_=qT[0]).then_inc(
                    qsem[0], 16
                )
                sync.dma_start(out=wqh_sb[:, :, :], in_=wqh[:, :, :]).then_inc(
                    wsem, 16
                )
                sync.dma_start(out=dT_sb[:, :, :], in_=dT[:, :, :]).then_inc(
                    dtsem, 16
                )
                sync.dma_start(out=bqh_sb[:, :], in_=bqh[:, :]).then_inc(bqsem, 16)
                for b in range(1, B):
                    sync.dma_start(out=qT_sb[b][:, :, :], in_=qT[b]).then_inc(
                        qsem[b], 16
                    )
                sync.dma_start(out=w2oh[:, :, :], in_=w2oh_in[:, :, :]).then_inc(
                    w2sem, 16
                )
                # tile 0 full, then tile 1 in two half-height pieces
                sync.wait_ge(vsem, v_out0)
                sync.dma_start(out=raw[0:128, :], in_=outt[0][:, :]).then_inc(
                    osem, 16
                )
                sync.wait_ge(vsem, v_out1a)
                sync.dma_start(out=raw[128:192, :], in_=outt[1][0:64, :]).then_inc(
                    osem, 16
                )
                sync.wait_ge(vsem, v_out1b)
                sync.dma_start(out=raw[192:256, :], in_=outt[1][64:128, :]).then_inc(
                    osem, 16
                )
                sync.wait_ge(osem, 48)

            @block.tensor
            def _(tensor):
                def qt_mm(b):
                    tensor.wait_ge(qsem[b], 16)
                    if b >= 2:
                        tensor.wait_ge(vsem, v_qtt[b - 2])
                    for j in range(NQC):
                        ins = nc.tensor.matmul(
                            qt_ps[b % 2][:, :],
                            wqh_sb[:, j, :],
                            qT_sb[b][:, j, :],
                            start=(j == 0),
                            stop=(j == NQC - 1),
                        )
                    ins.then_inc(psem, 1)

                tensor.wait_ge(wsem, 16)
                qt_mm(0)
                tensor.wait_ge(dtsem, 16)
                for j in range(NDC):
                    ins = nc.tensor.matmul(
                        ht_ps[:, :],
                        wqh_sb[:, NQC + j, :],
                        dT_sb[:, j, :],
                        start=(j == 0),
                        stop=(j == NDC - 1),
                    )
                ins.then_inc(psem, 1)
                for b in range(1, B):
                    qt_mm(b)
                tensor.wait_ge(w2sem, 16)
                for g in range(NCHUNK):
                    t, gt = divmod(g, GPT)
                    if t == 1 and gt == 0:
                        # tile 0's banks must be fully read before reuse
                        tensor.wait_ge(vsem, v_negmax0)
                        tensor.wait_ge(asem, a_exp0)
                    for pt in range(_parts_of(g)):
                        tensor.wait_ge(asem, a_tanh[(g, pt)])
                        for k in _part_ks(g, pt):
                            j, v, st, sp = _mm_plan(g, k)
                            ins = nc.tensor.matmul(
                                banks[j][32 * j:32 * (j + 1), :],
                                w2oh[:, v, :],
                                tin[g % 2][:, k * L_Q:(k + 1) * L_Q],
                                start=st,
                                stop=sp,
                                tile_position=(0, 32 * j),
                            )
                        ins.then_inc(psem, 1)

            @block.scalar
            def _(scalar):
                def exp_piece(t, j):
                    return nc.scalar.activation(
                        probs[t][32 * j:32 * (j + 1), :],
                        banks[j][32 * j:32 * (j + 1), :],
                        AF.Exp,
                        bias=negmax[t][32 * j:32 * (j + 1), :],
                        accum_out=sumexp[t][32 * j:32 * (j + 1), :],
                    )

                for g in range(NCHUNK):
                    for pt in range(_parts_of(g)):
                        scalar.wait_ge(vsem, v_adds[(g, pt)])
                        n = _parts_of(g)
                        w = (CH // n) * L_Q
                        nc.scalar.activation(
                            tin[g % 2][:, pt * w:(pt + 1) * w],
                            tin[g % 2][:, pt * w:(pt + 1) * w],
                            AF.Tanh,
                        ).then_inc(asem, 1)
                        if (g, pt) == (GPT, 0):
                            scalar.wait_ge(psem, p_last(GPT - 1))
                            scalar.wait_ge(vsem, v_negmax0)
                            for j in range(4):
                                ins = exp_piece(0, j)
                            ins.then_inc(asem, 1)
                        if (g, pt) == (NCHUNK - 1, 1):
                            # groups 0,1 of tile 1 completed at chunk 6
                            scalar.wait_ge(vsem, v_negmax1a)
                            for j in range(2):
                                ins = exp_piece(1, j)
                            ins.then_inc(asem, 1)
                scalar.wait_ge(psem, p_last(NCHUNK - 1))
                scalar.wait_ge(vsem, v_negmax1b)
                for j in range(2, 4):
                    ins = exp_piece(1, j)
                ins.then_inc(asem, 1)

            @block.vector
            def _(vector):
                def negmax_piece(t, j):
                    return nc.vector.tensor_reduce(
                        negmax[t][32 * j:32 * (j + 1), :],
                        banks[j][32 * j:32 * (j + 1), :],
                        axis=AX.X, op=ALU.max, negate=True,
                    )

                def scale_rows(t, lo, hi):
                    nc.vector.reciprocal(
                        rsum[t][lo:hi, :], sumexp[t][lo:hi, :]
                    )
                    vector.drain()
                    return nc.vector.tensor_scalar_mul(
                        outt[t][lo:hi, :], probs[t][lo:hi, :], rsum[t][lo:hi, :]
                    )

                vector.wait_ge(psem, p_qt[0])
                nc.vector.tensor_copy(qtt[0][:, :], qt_ps[0][:, :]).then_inc(
                    vsem, 1
                )
                vector.wait_ge(psem, p_ht)
                vector.wait_ge(bqsem, 16)
                nc.vector.tensor_scalar_add(
                    biasc[:, :], ht_ps[:, :], bqh_sb[:, :]
                ).then_inc(vsem, 1)
                vector.drain()
                for b in range(1, B):
                    vector.wait_ge(psem, p_qt[b])
                    nc.vector.tensor_copy(
                        qtt[b][:, :], qt_ps[b % 2][:, :]
                    ).then_inc(vsem, 1)
                for g in range(NCHUNK):
                    b = g // CPB
                    for pt in range(_parts_of(g)):
                        if g >= 2:
                            vector.wait_ge(psem, p_last(g - 2))
                        for k in _part_ks(g, pt):
                            ci = b * A_PER + (g % CPB) * CH + k
                            ins = nc.vector.tensor_scalar_add(
                                tin[g % 2][:, k * L_Q:(k + 1) * L_Q],
                                qtt[b][:, :],
                                biasc[:, ci:ci + 1],
                            )
                        ins.then_inc(vsem, 1)
                    if g == GPT:
                        vector.wait_ge(psem, p_last(GPT - 1))
                        for j in range(4):
                            ins = negmax_piece(0, j)
                        ins.then_inc(vsem, 1)
                    if g == GPT + 1:
                        vector.wait_ge(asem, a_exp0)
                        scale_rows(0, 0, 128).then_inc(vsem, 1)
                    if g == NCHUNK - 2:
                        # tile-1 groups 0,1 complete after chunk 5
                        vector.wait_ge(psem, p_last(NCHUNK - 3))
                        for j in range(2):
                            ins = negmax_piece(1, j)
                        ins.then_inc(vsem, 1)
                vector.wait_ge(psem, p_last(NCHUNK - 1))
                for j in range(2, 4):
                    ins = negmax_piece(1, j)
                ins.then_inc(vsem, 1)
                vector.wait_ge(asem, a_exp1a)
                scale_rows(1, 0, 64).then_inc(vsem, 1)
                vector.wait_ge(asem, a_exp1b)
                scale_rows(1, 64, 128).then_inc(vsem, 1)

    return nc


def _get_program():
    if "nc" not in _CACHE:
        _CACHE["nc"] = build_program()
    return _CACHE["nc"]


def _pmajor(a, nchunks):
    """(nchunks*128, X) -> (128, nchunks, X) partition-major layout."""
    x = a.reshape(nchunks, 128, a.shape[-1])
    return np.ascontiguousarray(x.transpose(1, 0, 2))


def _make_in_maps(inputs):
    import ml_dtypes

    query = np.asarray(inputs["query"], dtype=np.float32)
    decoder_states = np.asarray(inputs["decoder_states"], dtype=np.float32)
    Wq = np.asarray(inputs["Wq"], dtype=np.float32)
    Wh = np.asarray(inputs["Wh"], dtype=np.float32)
    wqh = _pmajor(np.vstack([Wq, Wh]), NWC)
    w2v = np.asarray(inputs["w2"], np.float32).reshape(H)
    w2oh = np.zeros((H, 32, 32), dtype=np.float32)
    w2oh[:, np.arange(32), np.arange(32)] = w2v[:, None]
    w2oh = w2oh.astype(ml_dtypes.bfloat16)
    bqh = np.ascontiguousarray(
        (np.asarray(inputs["bq"], np.float32)
         + np.asarray(inputs["bh"], np.float32)).reshape(H, 1)
    )
    qTf = query.transpose(1, 2, 0)  # (B, Q, L_q)
    qT = np.stack([_pmajor(qTf[b], NQC) for b in range(B)])
    in_maps = []
    for c in range(N_CORES):
        dslice = decoder_states[c * A_PER:(c + 1) * A_PER]
        # (D, B*A): column (b*A + a) holds decoder_states[a, b, :]
        dT = _pmajor(
            dslice.transpose(2, 1, 0).reshape(D_SIZE, NAB), NDC
        )
        in_maps.append({
            "qT": qT,
            "dT": dT,
            "wqh": wqh,
            "w2oh": w2oh,
            "bqh": bqh,
        })
    return in_maps


def kernel(query, decoder_states, query_mask, Wq, bq, Wh, bh, w2, b2):
    from concourse.bass_utils import run_bass_kernel_spmd

    mask = np.asarray(query_mask)
    nc = _get_program()
    in_maps = _make_in_maps({
        "query": query, "decoder_states": decoder_states,
        "Wq": Wq, "Wh": Wh, "w2": w2, "bq": bq, "bh": bh,
    })
    res = run_bass_kernel_spmd(nc, in_maps, list(range(N_CORES))).results
    perm = _row_perm()  # (A_PER, B) -> raw row
    out = np.empty((L_A, B, L_Q), dtype=np.float32)
    for c in range(N_CORES):
        out[c * A_PER:(c + 1) * A_PER] = res[c]["raw"][perm, :]

    if not mask.all():
        # exact post-exp masking + renormalization, host-side
        m = mask.T.astype(np.float32)  # (B, L_q)
        out = out * m[None, :, :]
        out = out / out.sum(axis=-1, keepdims=True)
    return out


# revision 20
# speedup vs baseline: 1.8464x; 1.0353x over previous
"""PointerNet attention scoring kernel for Trainium2 (8 NeuronCores).

Computes, for full inputs:
    q_t = query @ Wq + bq                      # (L_q, B, H)
    h_t = decoder_states @ Wh + bh             # (L_a, B, H)
    s[a,q,b] = sum_h tanh(q_t[q,b,h] + h_t[a,b,h]) * w2[h] (+ b2)
    out[a,b,q] = softmax_q(s[a,q,b])  (mask applied post-exp; ones here)

Sharding: data-parallel over L_a (512 -> 8 x 64). Each core receives the
full (host-pre-arranged, partition-major) query / weights and its
decoder_states slice, and produces a row-permuted (256, 512) block that
the host scatters into the (64, B, L_q) output slice. b2 is dropped
(softmax-invariant); the query mask, if not all ones, is applied
host-side (exactly). Host prep is layout-only - all FLOPs stay on
device.

Per-core on-chip pipeline (raw Bass, explicit semaphores - the walrus
build here only accepts one embedded sync-wait per instruction, so Tile
is unusable and all cross-engine waits are standalone wait_ge):
  - H=128 on partitions. q_tT[h,q] per b and bias columns
    h_tT[h,(b,a)]+bq+bh from small fp32 PE matmuls; stored bf16/f32.
  - Main loop, 8 chunks of CH=32 (a,b) pairs (first/last chunk split in
    two for pipeline ramp), bf16 datapath: DVE tensor_scalar_add
    broadcasts a bias column over q; one in-place ScalarE Tanh per
    chunk-part (ScalarE is the roofline: 16.8M elems / 128 lanes /
    1.2 GHz ~= 109 us); PE reduces each pair with a one-hot-scaled bf16
    w2 stationary ([128,32], w2 in column v) at tile_position (0,32j),
    accumulating into PSUM partition 32j+v of per-column-group banks
    (the 31 zero stationary columns add exact +0.0; bf16 matvecs are
    single-pass where fp32 would be two).
  - Scores tile 0 interleaves its matvecs over all 4 column-groups
    (4-way PE concurrency); its softmax hides under the next tanh.
    Tile 1 fills groups {0,1} during chunks 4-5 and {2,3} during 6-7
    (2-way concurrency) so half its softmax also hides under tanh and
    only groups 2,3 drain at the kernel tail.
  - Softmax over q (free axis), fp32, per 32-row bank piece: DVE
    negated max, ScalarE Exp with bias=-max and fused row-sum accum,
    DVE reciprocal + scale, 128/256 KB output DMAs.
"""

import numpy as np

L_Q, L_A, B = 512, 512, 4
Q_SIZE, D_SIZE, H = 256, 512, 128
N_CORES = 8
A_PER = L_A // N_CORES  # 64
CH = 32                 # (a,b) pairs per tanh chunk
NCHUNK = (A_PER * B) // CH          # 8
NTILE = (A_PER * B) // 128          # 2 scores tiles of 128 pair-rows
NAB = A_PER * B                     # 256 pair rows
GPT = 128 // CH                     # 4 chunks per scores tile
CPB = A_PER // CH                   # 2 chunks per batch entry
NQC = Q_SIZE // 128                 # 2 contraction chunks for q_t
NDC = D_SIZE // 128                 # 4 contraction chunks for h_t
NWC = NQC + NDC

_CACHE = {}


def _parts_of(g):
    return 4 if g in (0, NCHUNK - 1) else 1


def _part_ks(g, pt):
    n = _parts_of(g)
    lo = pt * (CH // n)
    return range(lo, lo + CH // n)


def _mm_plan(g, k):
    """(bank j, one-hot column v, start, stop) for pair-block k of chunk g."""
    t, gt = divmod(g, GPT)
    if t == 0:
        j = k % 4
        v = 8 * gt + k // 4
        return j, v, (gt == 0 and k < 4), (gt == GPT - 1 and k >= CH - 4)
    j = 2 * (gt // 2) + k % 2
    v = 16 * (gt % 2) + k // 2
    return j, v, (gt % 2 == 0 and k < 2), (gt % 2 == 1 and k >= CH - 2)


def _row_perm():
    """perm[a, b] = raw row index holding out[a, b, :]."""
    perm = np.empty((A_PER, B), dtype=np.int64)
    for g in range(NCHUNK):
        t = g // GPT
        b = g // CPB
        for k in range(CH):
            a = (g % CPB) * CH + k
            j, v, _, _ = _mm_plan(g, k)
            perm[a, b] = t * 128 + 32 * j + v
    return perm


def build_program():
    from contextlib import ExitStack

    import concourse.bass as bass
    from concourse import mybir

    f32 = mybir.dt.float32
    bf16 = mybir.dt.bfloat16
    AF = mybir.ActivationFunctionType
    ALU = mybir.AluOpType
    AX = mybir.AxisListType

    nc = bass.Bass()
    qT = nc.declare_dram_parameter("qT", [B, 128, NQC, L_Q], f32, isOutput=False)
    dT = nc.declare_dram_parameter("dT", [128, NDC, NAB], f32, isOutput=False)
    wqh = nc.declare_dram_parameter("wqh", [128, NWC, H], f32, isOutput=False)
    w2oh_in = nc.declare_dram_parameter("w2oh", [H, 32, 32], bf16, isOutput=False)
    bqh = nc.declare_dram_parameter("bqh", [H, 1], f32, isOutput=False)
    raw = nc.declare_dram_parameter("raw", [NAB, L_Q], f32, isOutput=True)

    with ExitStack() as ctx:
        _n = [0]

        def sb(shape, dt=f32):
            _n[0] += 1
            return ctx.enter_context(nc.sbuf_tensor(f"sb{_n[0]}", shape, dt))

        def ps(shape):
            _n[0] += 1
            return ctx.enter_context(nc.psum_tensor(f"ps{_n[0]}", shape, f32))

        wqh_sb = sb([128, NWC, H])
        w2oh = sb([128, 32, 32], bf16)
        bqh_sb = sb([128, 1])
        qT_sb = [sb([128, NQC, L_Q]) for _ in range(B)]
        dT_sb = sb([128, NDC, NAB])
        qtt = [sb([128, L_Q], bf16) for _ in range(B)]
        biasc = sb([128, NAB])  # fp32: tensor_scalar scalar1 must be f32
        tin = [sb([128, CH * L_Q], bf16) for _ in range(2)]
        probs = [sb([128, L_Q]) for _ in range(NTILE)]
        outt = [sb([128, L_Q]) for _ in range(NTILE)]
        sc0 = sb([128, L_Q])  # tile-0 scores gathered from the 4 banks
        negmax = [sb([128, 1]) for _ in range(NTILE)]
        sumexp = [sb([128, 1]) for _ in range(NTILE)]
        rsum = [sb([128, 1]) for _ in range(NTILE)]

        qt_ps = [ps([128, L_Q]) for _ in range(2)]
        ht_ps = ps([128, NAB])
        banks = [ps([128, L_Q]) for _ in range(4)]  # per col-group scores

        wsem = ctx.enter_context(nc.semaphore("wsem"))
        qsem = [ctx.enter_context(nc.semaphore(f"qsem{b}")) for b in range(B)]
        dtsem = ctx.enter_context(nc.semaphore("dtsem"))
        bqsem = ctx.enter_context(nc.semaphore("bqsem"))
        w2sem = ctx.enter_context(nc.semaphore("w2sem"))
        psem = ctx.enter_context(nc.semaphore("psem"))
        asem = ctx.enter_context(nc.semaphore("asem"))
        vsem = ctx.enter_context(nc.semaphore("vsem"))
        osem = ctx.enter_context(nc.semaphore("osem"))

        # --- semaphore milestones (mirror each engine's program order)
        # psem: qt b0 (1), ht (2), qt b1..b3 (3..5), then per chunk-part
        pc = 0
        pc += 1
        p_qt = {0: pc}
        pc += 1
        p_ht = pc
        for b in range(1, B):
            pc += 1
            p_qt[b] = pc
        p_chunk = {}
        for g in range(NCHUNK):
            for pt in range(_parts_of(g)):
                pc += 1
                p_chunk[(g, pt)] = pc

        def p_last(g):
            return p_chunk[(g, _parts_of(g) - 1)]

        # asem: tanh per chunk-part; exp0 (4 pieces) after tanh(4);
        # exp1 pieces {0,1} after tanh(7,0); pieces {2,3} at the end
        ac = 0
        a_tanh = {}
        for g in range(NCHUNK):
            for pt in range(_parts_of(g)):
                ac += 1
                a_tanh[(g, pt)] = ac
                if (g, pt) == (GPT, 0):
                    ac += 1
                    a_exp0 = ac
                if (g, pt) == (NCHUNK - 1, 1):
                    ac += 1
                    a_exp1a = ac
        ac += 1
        a_exp1b = ac

        # vsem: qtt0 (1), bias (2), qtt1..3 (3..5), per chunk-part adds,
        # plus woven softmax steps
        vc = 0
        vc += 1
        v_qtt = {0: vc}
        vc += 1
        v_bias = vc
        v_adds = {}
        for g in range(NCHUNK):
            if g == 2:
                for b in range(1, B):
                    vc += 1
                    v_qtt[b] = vc
            for pt in range(_parts_of(g)):
                vc += 1
                v_adds[(g, pt)] = vc
            if g == GPT:
                vc += 1
                v_negmax0 = vc
            if g == GPT + 1:
                vc += 1
                v_out0 = vc
            if g == NCHUNK - 2:
                vc += 1
                v_negmax1a = vc
        vc += 1
        v_negmax1b = vc
        vc += 1
        v_out1a = vc
        vc += 1
        v_out1b = vc

        with nc.Block() as block:

            @block.sync
            def _(sync):
                for h in range(2):
                    sync.dma_start(
                        out=dT_sb[:, 2 * h:2 * (h + 1), :],
                        in_=dT[:, 2 * h:2 * (h + 1), :],
                    ).then_inc(dtsem, 16)
                for h in range(2):
                    sync.dma_start(
                        out=qT_sb[0][:, h, :], in_=qT[0, :, h, :]
                    ).then_inc(qsem[0], 16)
                sync.dma_start(out=wqh_sb[:, :, :], in_=wqh[:, :, :]).then_inc(
                    wsem, 16
                )
                sync.dma_start(out=bqh_sb[:, :], in_=bqh[:, :]).then_inc(bqsem, 16)
                for b in range(1, B):
                    sync.dma_start(out=qT_sb[b][:, :, :], in_=qT[b]).then_inc(
                        qsem[b], 16
                    )
                sync.dma_start(out=w2oh[:, :, :], in_=w2oh_in[:, :, :]).then_inc(
                    w2sem, 16
                )
                # tile 0 full, then tile 1 in two half-height pieces
                sync.wait_ge(vsem, v_out0)
                sync.dma_start(out=raw[0:128, :], in_=outt[0][:, :]).then_inc(
                    osem, 16
                )
                sync.wait_ge(vsem, v_out1a)
                sync.dma_start(out=raw[128:192, :], in_=outt[1][0:64, :]).then_inc(
                    osem, 16
                )
                sync.wait_ge(vsem, v_out1b)
                sync.dma_start(out=raw[192:256, :], in_=outt[1][64:128, :]).then_inc(
                    osem, 16
                )
                sync.wait_ge(osem, 48)

            @block.tensor
            def _(tensor):
                def qt_mm(b):
                    tensor.wait_ge(qsem[b], 32 if b == 0 else 16)
                    if b >= 2:
                        tensor.wait_ge(vsem, v_qtt[b - 2])
                    for j in range(NQC):
                        ins = nc.tensor.matmul(
                            qt_ps[b % 2][:, :],
                            wqh_sb[:, j, :],
                            qT_sb[b][:, j, :],
                            start=(j == 0),
                            stop=(j == NQC - 1),
                        )
                    ins.then_inc(psem, 1)

                tensor.wait_ge(wsem, 16)
                qt_mm(0)
                tensor.wait_ge(dtsem, 32)
                for j in range(NDC):
                    ins = nc.tensor.matmul(
                        ht_ps[:, :],
                        wqh_sb[:, NQC + j, :],
                        dT_sb[:, j, :],
                        start=(j == 0),
                        stop=(j == NDC - 1),
                    )
                ins.then_inc(psem, 1)
                for b in range(1, B):
                    qt_mm(b)
                tensor.wait_ge(w2sem, 16)
                for g in range(NCHUNK):
                    t, gt = divmod(g, GPT)
                    if t == 1 and gt == 0:
                        # tile 0's banks are free once the DVE gather ran
                        tensor.wait_ge(vsem, v_negmax0)
                    for pt in range(_parts_of(g)):
                        tensor.wait_ge(asem, a_tanh[(g, pt)])
                        for k in _part_ks(g, pt):
                            j, v, st, sp = _mm_plan(g, k)
                            ins = nc.tensor.matmul(
                                banks[j][32 * j:32 * (j + 1), :],
                                w2oh[:, v, :],
                                tin[g % 2][:, k * L_Q:(k + 1) * L_Q],
                                start=st,
                                stop=sp,
                                tile_position=(0, 32 * j),
                            )
                        ins.then_inc(psem, 1)

            @block.scalar
            def _(scalar):
                def exp_piece(t, j):
                    return nc.scalar.activation(
                        probs[t][32 * j:32 * (j + 1), :],
                        banks[j][32 * j:32 * (j + 1), :],
                        AF.Exp,
                        bias=negmax[t][32 * j:32 * (j + 1), :],
                        accum_out=sumexp[t][32 * j:32 * (j + 1), :],
                    )

                for g in range(NCHUNK):
                    for pt in range(_parts_of(g)):
                        scalar.wait_ge(vsem, v_adds[(g, pt)])
                        n = _parts_of(g)
                        w = (CH // n) * L_Q
                        nc.scalar.activation(
                            tin[g % 2][:, pt * w:(pt + 1) * w],
                            tin[g % 2][:, pt * w:(pt + 1) * w],
                            AF.Tanh,
                        ).then_inc(asem, 1)
                        if (g, pt) == (GPT, 0):
                            scalar.wait_ge(vsem, v_negmax0)
                            nc.scalar.activation(
                                probs[0][:, :],
                                sc0[:, :],
                                AF.Exp,
                                bias=negmax[0][:, :],
                                accum_out=sumexp[0][:, :],
                            ).then_inc(asem, 1)
                        if (g, pt) == (NCHUNK - 1, 1):
                            # groups 0,1 of tile 1 completed at chunk 6
                            scalar.wait_ge(vsem, v_negmax1a)
                            for j in range(2):
                                ins = exp_piece(1, j)
                            ins.then_inc(asem, 1)
                scalar.wait_ge(psem, p_last(NCHUNK - 1))
                scalar.wait_ge(vsem, v_negmax1b)
                for j in range(2, 4):
                    ins = exp_piece(1, j)
                ins.then_inc(asem, 1)

            @block.vector
            def _(vector):
                def negmax_piece(t, j):
                    return nc.vector.tensor_reduce(
                        negmax[t][32 * j:32 * (j + 1), :],
                        banks[j][32 * j:32 * (j + 1), :],
                        axis=AX.X, op=ALU.max, negate=True,
                    )

                def scale_rows(t, lo, hi):
                    nc.vector.reciprocal(
                        rsum[t][lo:hi, :], sumexp[t][lo:hi, :]
                    )
                    vector.drain()
                    return nc.vector.tensor_scalar_mul(
                        outt[t][lo:hi, :], probs[t][lo:hi, :], rsum[t][lo:hi, :]
                    )

                vector.wait_ge(psem, p_qt[0])
                nc.vector.tensor_copy(qtt[0][:, :], qt_ps[0][:, :]).then_inc(
                    vsem, 1
                )
                vector.wait_ge(psem, p_ht)
                vector.wait_ge(bqsem, 16)
                nc.vector.tensor_scalar_add(
                    biasc[:, :], ht_ps[:, :], bqh_sb[:, :]
                ).then_inc(vsem, 1)
                vector.drain()
                for g in range(NCHUNK):
                    if g == 2:
                        for b in range(1, B):
                            vector.wait_ge(psem, p_qt[b])
                            nc.vector.tensor_copy(
                                qtt[b][:, :], qt_ps[b % 2][:, :]
                            ).then_inc(vsem, 1)
                    b = g // CPB
                    for pt in range(_parts_of(g)):
                        if g >= 2:
                            vector.wait_ge(psem, p_last(g - 2))
                        for k in _part_ks(g, pt):
                            ci = b * A_PER + (g % CPB) * CH + k
                            ins = nc.vector.tensor_scalar_add(
                                tin[g % 2][:, k * L_Q:(k + 1) * L_Q],
                                qtt[b][:, :],
                                biasc[:, ci:ci + 1],
                            )
                        ins.then_inc(vsem, 1)
                    if g == GPT:
                        vector.wait_ge(psem, p_last(GPT - 1))
                        for j in range(4):
                            nc.vector.tensor_copy(
                                sc0[:, :][32 * j:32 * (j + 1), :],
                                banks[j][32 * j:32 * (j + 1), :],
                            )
                        vector.drain()
                        nc.vector.tensor_reduce(
                            negmax[0][:, :], sc0[:, :],
                            axis=AX.X, op=ALU.max, negate=True,
                        ).then_inc(vsem, 1)
                    if g == GPT + 1:
                        vector.wait_ge(asem, a_exp0)
                        scale_rows(0, 0, 128).then_inc(vsem, 1)
                    if g == NCHUNK - 2:
                        # tile-1 groups 0,1 complete after chunk 5
                        vector.wait_ge(psem, p_last(NCHUNK - 3))
                        for j in range(2):
                            ins = negmax_piece(1, j)
                        ins.then_inc(vsem, 1)
                vector.wait_ge(psem, p_last(NCHUNK - 1))
                for j in range(2, 4):
                    ins = negmax_piece(1, j)
                ins.then_inc(vsem, 1)
                vector.wait_ge(asem, a_exp1a)
                scale_rows(1, 0, 64).then_inc(vsem, 1)
                vector.wait_ge(asem, a_exp1b)
                scale_rows(1, 64, 128).then_inc(vsem, 1)

    return nc


def _get_program():
    if "nc" not in _CACHE:
        _CACHE["nc"] = build_program()
    return _CACHE["nc"]


def _pmajor(a, nchunks):
    """(nchunks*128, X) -> (128, nchunks, X) partition-major layout."""
    x = a.reshape(nchunks, 128, a.shape[-1])
    return np.ascontiguousarray(x.transpose(1, 0, 2))


def _make_in_maps(inputs):
    import ml_dtypes

    query = np.asarray(inputs["query"], dtype=np.float32)
    decoder_states = np.asarray(inputs["decoder_states"], dtype=np.float32)
    Wq = np.asarray(inputs["Wq"], dtype=np.float32)
    Wh = np.asarray(inputs["Wh"], dtype=np.float32)
    wqh = _pmajor(np.vstack([Wq, Wh]), NWC)
    w2v = np.asarray(inputs["w2"], np.float32).reshape(H)
    w2oh = np.zeros((H, 32, 32), dtype=np.float32)
    w2oh[:, np.arange(32), np.arange(32)] = w2v[:, None]
    w2oh = w2oh.astype(ml_dtypes.bfloat16)
    bqh = np.ascontiguousarray(
        (np.asarray(inputs["bq"], np.float32)
         + np.asarray(inputs["bh"], np.float32)).reshape(H, 1)
    )
    qTf = query.transpose(1, 2, 0)  # (B, Q, L_q)
    qT = np.stack([_pmajor(qTf[b], NQC) for b in range(B)])
    in_maps = []
    for c in range(N_CORES):
        dslice = decoder_states[c * A_PER:(c + 1) * A_PER]
        # (D, B*A): column (b*A + a) holds decoder_states[a, b, :]
        dT = _pmajor(
            dslice.transpose(2, 1, 0).reshape(D_SIZE, NAB), NDC
        )
        in_maps.append({
            "qT": qT,
            "dT": dT,
            "wqh": wqh,
            "w2oh": w2oh,
            "bqh": bqh,
        })
    return in_maps


def kernel(query, decoder_states, query_mask, Wq, bq, Wh, bh, w2, b2):
    from concourse.bass_utils import run_bass_kernel_spmd

    mask = np.asarray(query_mask)
    nc = _get_program()
    in_maps = _make_in_maps({
        "query": query, "decoder_states": decoder_states,
        "Wq": Wq, "Wh": Wh, "w2": w2, "bq": bq, "bh": bh,
    })
    res = run_bass_kernel_spmd(nc, in_maps, list(range(N_CORES))).results
    perm = _row_perm()  # (A_PER, B) -> raw row
    out = np.empty((L_A, B, L_Q), dtype=np.float32)
    for c in range(N_CORES):
        out[c * A_PER:(c + 1) * A_PER] = res[c]["raw"][perm, :]

    if not mask.all():
        # exact post-exp masking + renormalization, host-side
        m = mask.T.astype(np.float32)  # (B, L_q)
        out = out * m[None, :, :]
        out = out / out.sum(axis=-1, keepdims=True)
    return out


# revision 22
# speedup vs baseline: 1.8549x; 1.0046x over previous
"""PointerNet attention scoring kernel for Trainium2 (8 NeuronCores).

Computes, for full inputs:
    q_t = query @ Wq + bq                      # (L_q, B, H)
    h_t = decoder_states @ Wh + bh             # (L_a, B, H)
    s[a,q,b] = sum_h tanh(q_t[q,b,h] + h_t[a,b,h]) * w2[h] (+ b2)
    out[a,b,q] = softmax_q(s[a,q,b])  (mask applied post-exp; ones here)

Sharding: data-parallel over L_a (512 -> 8 x 64). Each core receives the
full (host-pre-arranged, partition-major) query / weights and its
decoder_states slice, and produces a row-permuted (256, 512) block that
the host scatters into the (64, B, L_q) output slice. b2 is dropped
(softmax-invariant); the query mask, if not all ones, is applied
host-side (exactly). Host prep is layout-only - all FLOPs stay on
device.

Per-core on-chip pipeline (raw Bass, explicit semaphores - the walrus
build here only accepts one embedded sync-wait per instruction, so Tile
is unusable and all cross-engine waits are standalone wait_ge):
  - H=128 on partitions. q_tT[h,q] per b and bias columns
    h_tT[h,(b,a)]+bq+bh from small fp32 PE matmuls; stored bf16/f32.
  - Main loop, 8 chunks of CH=32 (a,b) pairs (first/last chunk split in
    two for pipeline ramp), bf16 datapath: DVE tensor_scalar_add
    broadcasts a bias column over q; one in-place ScalarE Tanh per
    chunk-part (ScalarE is the roofline: 16.8M elems / 128 lanes /
    1.2 GHz ~= 109 us); PE reduces each pair with a one-hot-scaled bf16
    w2 stationary ([128,32], w2 in column v) at tile_position (0,32j),
    accumulating into PSUM partition 32j+v of per-column-group banks
    (the 31 zero stationary columns add exact +0.0; bf16 matvecs are
    single-pass where fp32 would be two).
  - Scores tile 0 interleaves its matvecs over all 4 column-groups
    (4-way PE concurrency); its softmax hides under the next tanh.
    Tile 1 fills groups {0,1} during chunks 4-5 and {2,3} during 6-7
    (2-way concurrency) so half its softmax also hides under tanh and
    only groups 2,3 drain at the kernel tail.
  - Softmax over q (free axis), fp32, per 32-row bank piece: DVE
    negated max, ScalarE Exp with bias=-max and fused row-sum accum,
    DVE reciprocal + scale, 128/256 KB output DMAs.
"""

import numpy as np

L_Q, L_A, B = 512, 512, 4
Q_SIZE, D_SIZE, H = 256, 512, 128
N_CORES = 8
A_PER = L_A // N_CORES  # 64
CH = 32                 # (a,b) pairs per tanh chunk
NCHUNK = (A_PER * B) // CH          # 8
NTILE = (A_PER * B) // 128          # 2 scores tiles of 128 pair-rows
NAB = A_PER * B                     # 256 pair rows
GPT = 128 // CH                     # 4 chunks per scores tile
CPB = A_PER // CH                   # 2 chunks per batch entry
NQC = Q_SIZE // 128                 # 2 contraction chunks for q_t
NDC = D_SIZE // 128                 # 4 contraction chunks for h_t
NWC = NQC + NDC

_CACHE = {}


def _parts_of(g):
    return 4 if g in (0, NCHUNK - 1) else 1


def _part_ks(g, pt):
    n = _parts_of(g)
    lo = pt * (CH // n)
    return range(lo, lo + CH // n)


def _mm_plan(g, k):
    """(bank j, one-hot column v, start, stop) for pair-block k of chunk g."""
    t, gt = divmod(g, GPT)
    if t == 0:
        j = k % 4
        v = 8 * gt + k // 4
        return j, v, (gt == 0 and k < 4), (gt == GPT - 1 and k >= CH - 4)
    j = 2 * (gt // 2) + k % 2
    v = 16 * (gt % 2) + k // 2
    return j, v, (gt % 2 == 0 and k < 2), (gt % 2 == 1 and k >= CH - 2)


def _row_perm():
    """perm[a, b] = raw row index holding out[a, b, :]."""
    perm = np.empty((A_PER, B), dtype=np.int64)
    for g in range(NCHUNK):
        t = g // GPT
        b = g // CPB
        for k in range(CH):
            a = (g % CPB) * CH + k
            j, v, _, _ = _mm_plan(g, k)
            perm[a, b] = t * 128 + 32 * j + v
    return perm


def build_program():
    from contextlib import ExitStack

    import concourse.bass as bass
    from concourse import mybir

    f32 = mybir.dt.float32
    bf16 = mybir.dt.bfloat16
    AF = mybir.ActivationFunctionType
    ALU = mybir.AluOpType
    AX = mybir.AxisListType

    nc = bass.Bass()
    qT = nc.declare_dram_parameter("qT", [B, 128, NQC, L_Q], f32, isOutput=False)
    dT = nc.declare_dram_parameter("dT", [128, NDC, NAB], f32, isOutput=False)
    wqh = nc.declare_dram_parameter("wqh", [128, NWC, H], f32, isOutput=False)
    w2oh_in = nc.declare_dram_parameter("w2oh", [H, 32, 32], bf16, isOutput=False)
    bqh = nc.declare_dram_parameter("bqh", [H, 1], f32, isOutput=False)
    raw = nc.declare_dram_parameter("raw", [NAB, L_Q], f32, isOutput=True)

    with ExitStack() as ctx:
        _n = [0]

        def sb(shape, dt=f32):
            _n[0] += 1
            return ctx.enter_context(nc.sbuf_tensor(f"sb{_n[0]}", shape, dt))

        def ps(shape):
            _n[0] += 1
            return ctx.enter_context(nc.psum_tensor(f"ps{_n[0]}", shape, f32))

        wqh_sb = sb([128, NWC, H])
        w2oh = sb([128, 32, 32], bf16)
        bqh_sb = sb([128, 1])
        qT_sb = [sb([128, NQC, L_Q]) for _ in range(B)]
        dT_sb = sb([128, NDC, NAB])
        qtt = [sb([128, L_Q], bf16) for _ in range(B)]
        biasc = sb([128, NAB])  # fp32: tensor_scalar scalar1 must be f32
        tin = [sb([128, CH * L_Q], bf16) for _ in range(2)]
        probs = [sb([128, L_Q]) for _ in range(NTILE)]
        outt = [sb([128, L_Q]) for _ in range(NTILE)]
        sc0 = sb([128, L_Q])  # tile-0 scores gathered from the 4 banks
        negmax = [sb([128, 1]) for _ in range(NTILE)]
        sumexp = [sb([128, 1]) for _ in range(NTILE)]
        rsum = [sb([128, 1]) for _ in range(NTILE)]

        qt_ps = [ps([128, L_Q]) for _ in range(2)]
        ht_ps = ps([128, NAB])
        banks = [ps([128, L_Q]) for _ in range(4)]  # per col-group scores

        wsem = ctx.enter_context(nc.semaphore("wsem"))
        qsem = [ctx.enter_context(nc.semaphore(f"qsem{b}")) for b in range(B)]
        dtsem = ctx.enter_context(nc.semaphore("dtsem"))
        bqsem = ctx.enter_context(nc.semaphore("bqsem"))
        w2sem = ctx.enter_context(nc.semaphore("w2sem"))
        psem = ctx.enter_context(nc.semaphore("psem"))
        asem = ctx.enter_context(nc.semaphore("asem"))
        vsem = ctx.enter_context(nc.semaphore("vsem"))
        osem = ctx.enter_context(nc.semaphore("osem"))

        # --- semaphore milestones (mirror each engine's program order)
        # psem: qt b0 (1), ht (2), qt b1..b3 (3..5), then per chunk-part
        pc = 0
        pc += 1
        p_ht = pc
        p_qt = {}
        for b in range(B):
            pc += 1
            p_qt[b] = pc
        p_chunk = {}
        for g in range(NCHUNK):
            for pt in range(_parts_of(g)):
                pc += 1
                p_chunk[(g, pt)] = pc

        def p_last(g):
            return p_chunk[(g, _parts_of(g) - 1)]

        # asem: tanh per chunk-part; exp0 (4 pieces) after tanh(4);
        # exp1 pieces {0,1} after tanh(7,0); pieces {2,3} at the end
        ac = 0
        a_tanh = {}
        for g in range(NCHUNK):
            for pt in range(_parts_of(g)):
                ac += 1
                a_tanh[(g, pt)] = ac
                if (g, pt) == (GPT + 1, 0):
                    ac += 1
                    a_exp0 = ac
                if (g, pt) == (NCHUNK - 1, 1):
                    ac += 1
                    a_exp1a = ac
        ac += 1
        a_exp1b = ac

        # vsem: qtt0 (1), bias (2), qtt1..3 (3..5), per chunk-part adds,
        # plus woven softmax steps
        vc = 0
        vc += 1
        v_bias = vc
        vc += 1
        v_qtt = {0: vc}
        v_adds = {}
        for g in range(NCHUNK):
            if g == 2:
                for b in range(1, B):
                    vc += 1
                    v_qtt[b] = vc
            for pt in range(_parts_of(g)):
                vc += 1
                v_adds[(g, pt)] = vc
            if g == GPT:
                vc += 1
                v_negmax0 = vc
            if g == GPT + 2:
                vc += 1
                v_out0 = vc
            if g == NCHUNK - 2:
                vc += 1
                v_negmax1a = vc
        vc += 1
        v_negmax1b = vc
        vc += 1
        v_out1a = vc
        vc += 1
        v_out1b = vc

        with nc.Block() as block:

            @block.sync
            def _(sync):
                for h in range(NDC):
                    sync.dma_start(
                        out=dT_sb[:, h, :], in_=dT[:, h, :]
                    ).then_inc(dtsem, 16)
                sync.dma_start(out=wqh_sb[:, :, :], in_=wqh[:, :, :]).then_inc(
                    wsem, 16
                )
                for h in range(2):
                    sync.dma_start(
                        out=qT_sb[0][:, h, :], in_=qT[0, :, h, :]
                    ).then_inc(qsem[0], 16)
                sync.dma_start(out=bqh_sb[:, :], in_=bqh[:, :]).then_inc(bqsem, 16)
                for b in range(1, B):
                    sync.dma_start(out=qT_sb[b][:, :, :], in_=qT[b]).then_inc(
                        qsem[b], 16
                    )
                sync.dma_start(out=w2oh[:, :, :], in_=w2oh_in[:, :, :]).then_inc(
                    w2sem, 16
                )
                # tile 0 full, then tile 1 in two half-height pieces
                sync.wait_ge(vsem, v_out0)
                sync.dma_start(out=raw[0:128, :], in_=outt[0][:, :]).then_inc(
                    osem, 16
                )
                sync.wait_ge(vsem, v_out1a)
                sync.dma_start(out=raw[128:192, :], in_=outt[1][0:64, :]).then_inc(
                    osem, 16
                )
                sync.wait_ge(vsem, v_out1b)
                sync.dma_start(out=raw[192:256, :], in_=outt[1][64:128, :]).then_inc(
                    osem, 16
                )
                sync.wait_ge(osem, 48)

            @block.tensor
            def _(tensor):
                def qt_mm(b):
                    tensor.wait_ge(qsem[b], 32 if b == 0 else 16)
                    if b >= 2:
                        tensor.wait_ge(vsem, v_qtt[b - 2])
                    for j in range(NQC):
                        ins = nc.tensor.matmul(
                            qt_ps[b % 2][:, :],
                            wqh_sb[:, j, :],
                            qT_sb[b][:, j, :],
                            start=(j == 0),
                            stop=(j == NQC - 1),
                        )
                    ins.then_inc(psem, 1)

                tensor.wait_ge(wsem, 16)
                tensor.wait_ge(dtsem, 16 * NDC)
                for j in range(NDC):
                    ins = nc.tensor.matmul(
                        ht_ps[:, :],
                        wqh_sb[:, NQC + j, :],
                        dT_sb[:, j, :],
                        start=(j == 0),
                        stop=(j == NDC - 1),
                    )
                ins.then_inc(psem, 1)
                for b in range(B):
                    qt_mm(b)
                tensor.wait_ge(w2sem, 16)
                for g in range(NCHUNK):
                    t, gt = divmod(g, GPT)
                    if t == 1 and gt == 0:
                        # tile 0's banks are free once the DVE gather ran
                        tensor.wait_ge(vsem, v_negmax0)
                    for pt in range(_parts_of(g)):
                        tensor.wait_ge(asem, a_tanh[(g, pt)])
                        for k in _part_ks(g, pt):
                            j, v, st, sp = _mm_plan(g, k)
                            ins = nc.tensor.matmul(
                                banks[j][32 * j:32 * (j + 1), :],
                                w2oh[:, v, :],
                                tin[g % 2][:, k * L_Q:(k + 1) * L_Q],
                                start=st,
                                stop=sp,
                                tile_position=(0, 32 * j),
                            )
                        ins.then_inc(psem, 1)

            @block.scalar
            def _(scalar):
                def exp_piece(t, j):
                    return nc.scalar.activation(
                        probs[t][32 * j:32 * (j + 1), :],
                        banks[j][32 * j:32 * (j + 1), :],
                        AF.Exp,
                        bias=negmax[t][32 * j:32 * (j + 1), :],
                        accum_out=sumexp[t][32 * j:32 * (j + 1), :],
                    )

                for g in range(NCHUNK):
                    for pt in range(_parts_of(g)):
                        scalar.wait_ge(vsem, v_adds[(g, pt)])
                        n = _parts_of(g)
                        w = (CH // n) * L_Q
                        nc.scalar.activation(
                            tin[g % 2][:, pt * w:(pt + 1) * w],
                            tin[g % 2][:, pt * w:(pt + 1) * w],
                            AF.Tanh,
                        ).then_inc(asem, 1)
                        if (g, pt) == (GPT + 1, 0):
                            scalar.wait_ge(vsem, v_negmax0)
                            nc.scalar.activation(
                                probs[0][:, :],
                                sc0[:, :],
                                AF.Exp,
                                bias=negmax[0][:, :],
                                accum_out=sumexp[0][:, :],
                            ).then_inc(asem, 1)
                        if (g, pt) == (NCHUNK - 1, 1):
                            # groups 0,1 of tile 1 completed at chunk 6
                            scalar.wait_ge(vsem, v_negmax1a)
                            for j in range(2):
                                ins = exp_piece(1, j)
                            ins.then_inc(asem, 1)
                scalar.wait_ge(psem, p_last(NCHUNK - 1))
                scalar.wait_ge(vsem, v_negmax1b)
                for j in range(2, 4):
                    ins = exp_piece(1, j)
                ins.then_inc(asem, 1)

            @block.vector
            def _(vector):
                def negmax_piece(t, j):
                    return nc.vector.tensor_reduce(
                        negmax[t][32 * j:32 * (j + 1), :],
                        banks[j][32 * j:32 * (j + 1), :],
                        axis=AX.X, op=ALU.max, negate=True,
                    )

                def scale_rows(t, lo, hi):
                    nc.vector.reciprocal(
                        rsum[t][lo:hi, :], sumexp[t][lo:hi, :]
                    )
                    vector.drain()
                    return nc.vector.tensor_scalar_mul(
                        outt[t][lo:hi, :], probs[t][lo:hi, :], rsum[t][lo:hi, :]
                    )

                vector.wait_ge(psem, p_ht)
                vector.wait_ge(bqsem, 16)
                nc.vector.tensor_scalar_add(
                    biasc[:, :], ht_ps[:, :], bqh_sb[:, :]
                ).then_inc(vsem, 1)
                vector.wait_ge(psem, p_qt[0])
                nc.vector.tensor_copy(qtt[0][:, :], qt_ps[0][:, :]).then_inc(
                    vsem, 1
                )
                vector.drain()
                for g in range(NCHUNK):
                    if g == 2:
                        for b in range(1, B):
                            vector.wait_ge(psem, p_qt[b])
                            nc.vector.tensor_copy(
                                qtt[b][:, :], qt_ps[b % 2][:, :]
                            ).then_inc(vsem, 1)
                    b = g // CPB
                    for pt in range(_parts_of(g)):
                        if g >= 2:
                            vector.wait_ge(psem, p_last(g - 2))
                        for k in _part_ks(g, pt):
                            ci = b * A_PER + (g % CPB) * CH + k
                            ins = nc.vector.tensor_scalar_add(
                                tin[g % 2][:, k * L_Q:(k + 1) * L_Q],
                                qtt[b][:, :],
                                biasc[:, ci:ci + 1],
                            )
                        ins.then_inc(vsem, 1)
                    if g == GPT:
                        vector.wait_ge(psem, p_last(GPT - 1))
                        for j in range(4):
                            nc.vector.tensor_copy(
                                sc0[:, :][32 * j:32 * (j + 1), :],
                                banks[j][32 * j:32 * (j + 1), :],
                            )
                        vector.drain()
                        nc.vector.tensor_reduce(
                            negmax[0][:, :], sc0[:, :],
                            axis=AX.X, op=ALU.max, negate=True,
                        ).then_inc(vsem, 1)
                    if g == GPT + 2:
                        vector.wait_ge(asem, a_exp0)
                        scale_rows(0, 0, 128).then_inc(vsem, 1)
                    if g == NCHUNK - 2:
                        # tile-1 groups 0,1 complete after chunk 5
                        vector.wait_ge(psem, p_last(NCHUNK - 3))
                        for j in range(2):
                            ins = negmax_piece(1, j)
                        ins.then_inc(vsem, 1)
                vector.wait_ge(psem, p_last(NCHUNK - 1))
                for j in range(2, 4):
                    ins = negmax_piece(1, j)
                ins.then_inc(vsem, 1)
                vector.wait_ge(asem, a_exp1a)
                scale_rows(1, 0, 64).then_inc(vsem, 1)
                vector.wait_ge(asem, a_exp1b)
                scale_rows(1, 64, 128).then_inc(vsem, 1)

    return nc


def _get_program():
    if "nc" not in _CACHE:
        _CACHE["nc"] = build_program()
    return _CACHE["nc"]


def _pmajor(a, nchunks):
    """(nchunks*128, X) -> (128, nchunks, X) partition-major layout."""
    x = a.reshape(nchunks, 128, a.shape[-1])
    return np.ascontiguousarray(x.transpose(1, 0, 2))


def _make_in_maps(inputs):
    import ml_dtypes

    query = np.asarray(inputs["query"], dtype=np.float32)
    decoder_states = np.asarray(inputs["decoder_states"], dtype=np.float32)
    Wq = np.asarray(inputs["Wq"], dtype=np.float32)
    Wh = np.asarray(inputs["Wh"], dtype=np.float32)
    wqh = _pmajor(np.vstack([Wq, Wh]), NWC)
    w2v = np.asarray(inputs["w2"], np.float32).reshape(H)
    w2oh = np.zeros((H, 32, 32), dtype=np.float32)
    w2oh[:, np.arange(32), np.arange(32)] = w2v[:, None]
    w2oh = w2oh.astype(ml_dtypes.bfloat16)
    bqh = np.ascontiguousarray(
        (np.asarray(inputs["bq"], np.float32)
         + np.asarray(inputs["bh"], np.float32)).reshape(H, 1)
    )
    qTf = query.transpose(1, 2, 0)  # (B, Q, L_q)
    qT = np.stack([_pmajor(qTf[b], NQC) for b in range(B)])
    in_maps = []
    for c in range(N_CORES):
        dslice = decoder_states[c * A_PER:(c + 1) * A_PER]
        # (D, B*A): column (b*A + a) holds decoder_states[a, b, :]
        dT = _pmajor(
            dslice.transpose(2, 1, 0).reshape(D_SIZE, NAB), NDC
        )
        in_maps.append({
            "qT": qT,
            "dT": dT,
            "wqh": wqh,
            "w2oh": w2oh,
            "bqh": bqh,
        })
    return in_maps


def kernel(query, decoder_states, query_mask, Wq, bq, Wh, bh, w2, b2):
    from concourse.bass_utils import run_bass_kernel_spmd

    mask = np.asarray(query_mask)
    nc = _get_program()
    in_maps = _make_in_maps({
        "query": query, "decoder_states": decoder_states,
        "Wq": Wq, "Wh": Wh, "w2": w2, "bq": bq, "bh": bh,
    })
    res = run_bass_kernel_spmd(nc, in_maps, list(range(N_CORES))).results
    perm = _row_perm()  # (A_PER, B) -> raw row
    out = np.empty((L_A, B, L_Q), dtype=np.float32)
    for c in range(N_CORES):
        out[c * A_PER:(c + 1) * A_PER] = res[c]["raw"][perm, :]

    if not mask.all():
        # exact post-exp masking + renormalization, host-side
        m = mask.T.astype(np.float32)  # (B, L_q)
        out = out * m[None, :, :]
        out = out / out.sum(axis=-1, keepdims=True)
    return out


# revision 23
# speedup vs baseline: 1.8561x; 1.0007x over previous
"""PointerNet attention scoring kernel for Trainium2 (8 NeuronCores).

Computes, for full inputs:
    q_t = query @ Wq + bq                      # (L_q, B, H)
    h_t = decoder_states @ Wh + bh             # (L_a, B, H)
    s[a,q,b] = sum_h tanh(q_t[q,b,h] + h_t[a,b,h]) * w2[h] (+ b2)
    out[a,b,q] = softmax_q(s[a,q,b])  (mask applied post-exp; ones here)

Sharding: data-parallel over L_a (512 -> 8 x 64). Each core receives the
full (host-pre-arranged, partition-major) query / weights and its
decoder_states slice, and produces a row-permuted (256, 512) block that
the host scatters into the (64, B, L_q) output slice. b2 is dropped
(softmax-invariant); the query mask, if not all ones, is applied
host-side (exactly). Host prep is layout-only - all FLOPs stay on
device.

Per-core on-chip pipeline (raw Bass, explicit semaphores - the walrus
build here only accepts one embedded sync-wait per instruction, so Tile
is unusable and all cross-engine waits are standalone wait_ge):
  - H=128 on partitions. q_tT[h,q] per b and bias columns
    h_tT[h,(b,a)]+bq+bh from small fp32 PE matmuls; stored bf16/f32.
  - Main loop, 8 chunks of CH=32 (a,b) pairs (first/last chunk split in
    two for pipeline ramp), bf16 datapath: DVE tensor_scalar_add
    broadcasts a bias column over q; one in-place ScalarE Tanh per
    chunk-part (ScalarE is the roofline: 16.8M elems / 128 lanes /
    1.2 GHz ~= 109 us); PE reduces each pair with a one-hot-scaled bf16
    w2 stationary ([128,32], w2 in column v) at tile_position (0,32j),
    accumulating into PSUM partition 32j+v of per-column-group banks
    (the 31 zero stationary columns add exact +0.0; bf16 matvecs are
    single-pass where fp32 would be two).
  - Scores tile 0 interleaves its matvecs over all 4 column-groups
    (4-way PE concurrency); its softmax hides under the next tanh.
    Tile 1 fills groups {0,1} during chunks 4-5 and {2,3} during 6-7
    (2-way concurrency) so half its softmax also hides under tanh and
    only groups 2,3 drain at the kernel tail.
  - Softmax over q (free axis), fp32, per 32-row bank piece: DVE
    negated max, ScalarE Exp with bias=-max and fused row-sum accum,
    DVE reciprocal + scale, 128/256 KB output DMAs.
"""

import numpy as np

L_Q, L_A, B = 512, 512, 4
Q_SIZE, D_SIZE, H = 256, 512, 128
N_CORES = 8
A_PER = L_A // N_CORES  # 64
CH = 32                 # (a,b) pairs per tanh chunk
NCHUNK = (A_PER * B) // CH          # 8
NTILE = (A_PER * B) // 128          # 2 scores tiles of 128 pair-rows
NAB = A_PER * B                     # 256 pair rows
GPT = 128 // CH                     # 4 chunks per scores tile
CPB = A_PER // CH                   # 2 chunks per batch entry
NQC = Q_SIZE // 128                 # 2 contraction chunks for q_t
NDC = D_SIZE // 128                 # 4 contraction chunks for h_t
NWC = NQC + NDC

_CACHE = {}


def _parts_of(g):
    return 4 if g in (0, NCHUNK - 1) else 1


def _part_ks(g, pt):
    n = _parts_of(g)
    lo = pt * (CH // n)
    return range(lo, lo + CH // n)


def _mm_plan(g, k):
    """(bank j, one-hot column v, start, stop) for pair-block k of chunk g."""
    t, gt = divmod(g, GPT)
    if t == 0:
        j = k % 4
        v = 8 * gt + k // 4
        return j, v, (gt == 0 and k < 4), (gt == GPT - 1 and k >= CH - 4)
    j = 2 * (gt // 2) + k % 2
    v = 16 * (gt % 2) + k // 2
    return j, v, (gt % 2 == 0 and k < 2), (gt % 2 == 1 and k >= CH - 2)


def _row_perm():
    """perm[a, b] = raw row index holding out[a, b, :]."""
    perm = np.empty((A_PER, B), dtype=np.int64)
    for g in range(NCHUNK):
        t = g // GPT
        b = g // CPB
        for k in range(CH):
            a = (g % CPB) * CH + k
            j, v, _, _ = _mm_plan(g, k)
            perm[a, b] = t * 128 + 32 * j + v
    return perm


def build_program():
    from contextlib import ExitStack

    import concourse.bass as bass
    from concourse import mybir

    f32 = mybir.dt.float32
    bf16 = mybir.dt.bfloat16
    AF = mybir.ActivationFunctionType
    ALU = mybir.AluOpType
    AX = mybir.AxisListType

    nc = bass.Bass()
    qT = nc.declare_dram_parameter("qT", [B, 128, NQC, L_Q], f32, isOutput=False)
    dT = nc.declare_dram_parameter("dT", [128, NDC, NAB], f32, isOutput=False)
    wqh = nc.declare_dram_parameter("wqh", [128, NWC, H], f32, isOutput=False)
    w2oh_in = nc.declare_dram_parameter("w2oh", [H, 32, 32], bf16, isOutput=False)
    bqh = nc.declare_dram_parameter("bqh", [H, 1], f32, isOutput=False)
    raw = nc.declare_dram_parameter("raw", [NAB, L_Q], f32, isOutput=True)

    with ExitStack() as ctx:
        _n = [0]

        def sb(shape, dt=f32):
            _n[0] += 1
            return ctx.enter_context(nc.sbuf_tensor(f"sb{_n[0]}", shape, dt))

        def ps(shape):
            _n[0] += 1
            return ctx.enter_context(nc.psum_tensor(f"ps{_n[0]}", shape, f32))

        wqh_sb = sb([128, NWC, H])
        w2oh = sb([128, 32, 32], bf16)
        bqh_sb = sb([128, 1])
        qT_sb = [sb([128, NQC, L_Q]) for _ in range(B)]
        dT_sb = sb([128, NDC, NAB])
        qtt = [sb([128, L_Q], bf16) for _ in range(B)]
        biasc = sb([128, NAB])  # fp32: tensor_scalar scalar1 must be f32
        tin = [sb([128, CH * L_Q], bf16) for _ in range(2)]
        probs = [sb([128, L_Q]) for _ in range(NTILE)]
        outt = [sb([128, L_Q]) for _ in range(NTILE)]
        sc0 = sb([128, L_Q])  # tile-0 scores gathered from the 4 banks
        negmax = [sb([128, 1]) for _ in range(NTILE)]
        sumexp = [sb([128, 1]) for _ in range(NTILE)]
        rsum = [sb([128, 1]) for _ in range(NTILE)]

        qt_ps = [ps([128, L_Q]) for _ in range(2)]
        ht_ps = ps([128, L_Q])   # prep uses [:, :NAB]; later a score bank
        spare_ps = ps([128, L_Q])
        banks = [ps([128, L_Q]) for _ in range(4)]  # tile-0 col-group scores
        banks1 = [qt_ps[0], qt_ps[1], ht_ps, spare_ps]  # tile-1 reuses prep

        wsem = ctx.enter_context(nc.semaphore("wsem"))
        qsem = [ctx.enter_context(nc.semaphore(f"qsem{b}")) for b in range(B)]
        dtsem = ctx.enter_context(nc.semaphore("dtsem"))
        bqsem = ctx.enter_context(nc.semaphore("bqsem"))
        w2sem = ctx.enter_context(nc.semaphore("w2sem"))
        psem = ctx.enter_context(nc.semaphore("psem"))
        asem = ctx.enter_context(nc.semaphore("asem"))
        vsem = ctx.enter_context(nc.semaphore("vsem"))
        osem = ctx.enter_context(nc.semaphore("osem"))

        # --- semaphore milestones (mirror each engine's program order)
        # psem: qt b0 (1), ht (2), qt b1..b3 (3..5), then per chunk-part
        pc = 0
        pc += 1
        p_ht = pc
        p_qt = {}
        for b in range(B):
            pc += 1
            p_qt[b] = pc
        p_chunk = {}
        for g in range(NCHUNK):
            for pt in range(_parts_of(g)):
                pc += 1
                p_chunk[(g, pt)] = pc

        def p_last(g):
            return p_chunk[(g, _parts_of(g) - 1)]

        # asem: tanh per chunk-part; exp0 (4 pieces) after tanh(4);
        # exp1 pieces {0,1} after tanh(7,0); pieces {2,3} at the end
        ac = 0
        a_tanh = {}
        for g in range(NCHUNK):
            for pt in range(_parts_of(g)):
                ac += 1
                a_tanh[(g, pt)] = ac
                if (g, pt) == (GPT + 2, 0):
                    ac += 1
                    a_exp0 = ac
                if (g, pt) == (NCHUNK - 1, 1):
                    ac += 1
                    a_exp1a = ac
        ac += 1
        a_exp1b = ac

        # vsem: qtt0 (1), bias (2), qtt1..3 (3..5), per chunk-part adds,
        # plus woven softmax steps
        vc = 0
        vc += 1
        v_bias = vc
        vc += 1
        v_qtt = {0: vc}
        v_adds = {}
        for g in range(NCHUNK):
            if g == 2:
                for b in range(1, B):
                    vc += 1
                    v_qtt[b] = vc
            for pt in range(_parts_of(g)):
                vc += 1
                v_adds[(g, pt)] = vc
            if g == GPT + 2:
                vc += 1
                v_negmax0 = vc
            if g == NCHUNK - 1:
                vc += 1
                v_negmax1a = vc
                vc += 1
                v_out0 = vc
        vc += 1
        v_negmax1b = vc
        vc += 1
        v_out1a = vc
        vc += 1
        v_out1b = vc

        with nc.Block() as block:

            @block.sync
            def _(sync):
                for h in range(2):
                    sync.dma_start(
                        out=dT_sb[:, 2 * h:2 * (h + 1), :],
                        in_=dT[:, 2 * h:2 * (h + 1), :],
                    ).then_inc(dtsem, 16)
                sync.dma_start(out=wqh_sb[:, :, :], in_=wqh[:, :, :]).then_inc(
                    wsem, 16
                )
                sync.dma_start(out=qT_sb[0][:, :, :], in_=qT[0]).then_inc(
                    qsem[0], 16
                )
                sync.dma_start(out=bqh_sb[:, :], in_=bqh[:, :]).then_inc(bqsem, 16)
                for b in range(1, B):
                    sync.dma_start(out=qT_sb[b][:, :, :], in_=qT[b]).then_inc(
                        qsem[b], 16
                    )
                sync.dma_start(out=w2oh[:, :, :], in_=w2oh_in[:, :, :]).then_inc(
                    w2sem, 16
                )
                # tile 0 full, then tile 1 in two half-height pieces
                sync.wait_ge(vsem, v_out0)
                sync.dma_start(out=raw[0:128, :], in_=outt[0][:, :]).then_inc(
                    osem, 16
                )
                sync.wait_ge(vsem, v_out1a)
                sync.dma_start(out=raw[128:192, :], in_=outt[1][0:64, :]).then_inc(
                    osem, 16
                )
                sync.wait_ge(vsem, v_out1b)
                sync.dma_start(out=raw[192:256, :], in_=outt[1][64:128, :]).then_inc(
                    osem, 16
                )
                sync.wait_ge(osem, 48)

            @block.tensor
            def _(tensor):
                def qt_mm(b):
                    tensor.wait_ge(qsem[b], 16)
                    if b >= 2:
                        tensor.wait_ge(vsem, v_qtt[b - 2])
                    for j in range(NQC):
                        ins = nc.tensor.matmul(
                            qt_ps[b % 2][:, :],
                            wqh_sb[:, j, :],
                            qT_sb[b][:, j, :],
                            start=(j == 0),
                            stop=(j == NQC - 1),
                        )
                    ins.then_inc(psem, 1)

                tensor.wait_ge(wsem, 16)
                tensor.wait_ge(dtsem, 32)
                for j in range(NDC):
                    ins = nc.tensor.matmul(
                        ht_ps[:, :NAB],
                        wqh_sb[:, NQC + j, :],
                        dT_sb[:, j, :],
                        start=(j == 0),
                        stop=(j == NDC - 1),
                    )
                ins.then_inc(psem, 1)
                for b in range(B):
                    qt_mm(b)
                tensor.wait_ge(w2sem, 16)
                for g in range(NCHUNK):
                    t, gt = divmod(g, GPT)
                    if t == 1 and gt == 0:
                        # tile 1 reuses the (dead) prep banks
                        tensor.wait_ge(vsem, v_qtt[B - 1])
                    for pt in range(_parts_of(g)):
                        tensor.wait_ge(asem, a_tanh[(g, pt)])
                        for k in _part_ks(g, pt):
                            j, v, st, sp = _mm_plan(g, k)
                            bk = banks[j] if t == 0 else banks1[j]
                            ins = nc.tensor.matmul(
                                bk[32 * j:32 * (j + 1), :],
                                w2oh[:, v, :],
                                tin[g % 2][:, k * L_Q:(k + 1) * L_Q],
                                start=st,
                                stop=sp,
                                tile_position=(0, 32 * j),
                            )
                        ins.then_inc(psem, 1)

            @block.scalar
            def _(scalar):
                def exp_piece(t, j):
                    bk = banks[j] if t == 0 else banks1[j]
                    return nc.scalar.activation(
                        probs[t][32 * j:32 * (j + 1), :],
                        bk[32 * j:32 * (j + 1), :],
                        AF.Exp,
                        bias=negmax[t][32 * j:32 * (j + 1), :],
                        accum_out=sumexp[t][32 * j:32 * (j + 1), :],
                    )

                for g in range(NCHUNK):
                    for pt in range(_parts_of(g)):
                        scalar.wait_ge(vsem, v_adds[(g, pt)])
                        n = _parts_of(g)
                        w = (CH // n) * L_Q
                        nc.scalar.activation(
                            tin[g % 2][:, pt * w:(pt + 1) * w],
                            tin[g % 2][:, pt * w:(pt + 1) * w],
                            AF.Tanh,
                        ).then_inc(asem, 1)
                        if (g, pt) == (GPT + 2, 0):
                            scalar.wait_ge(vsem, v_negmax0)
                            nc.scalar.activation(
                                probs[0][:, :],
                                sc0[:, :],
                                AF.Exp,
                                bias=negmax[0][:, :],
                                accum_out=sumexp[0][:, :],
                            ).then_inc(asem, 1)
                        if (g, pt) == (NCHUNK - 1, 1):
                            # groups 0,1 of tile 1 completed at chunk 6
                            scalar.wait_ge(vsem, v_negmax1a)
                            for j in range(2):
                                ins = exp_piece(1, j)
                            ins.then_inc(asem, 1)
                scalar.wait_ge(psem, p_last(NCHUNK - 1))
                scalar.wait_ge(vsem, v_negmax1b)
                for j in range(2, 4):
                    ins = exp_piece(1, j)
                ins.then_inc(asem, 1)

            @block.vector
            def _(vector):
                def negmax_piece(t, j):
                    bk = banks[j] if t == 0 else banks1[j]
                    return nc.vector.tensor_reduce(
                        negmax[t][32 * j:32 * (j + 1), :],
                        bk[32 * j:32 * (j + 1), :],
                        axis=AX.X, op=ALU.max, negate=True,
                    )

                def scale_rows(t, lo, hi):
                    nc.vector.reciprocal(
                        rsum[t][lo:hi, :], sumexp[t][lo:hi, :]
                    )
                    vector.drain()
                    return nc.vector.tensor_scalar_mul(
                        outt[t][lo:hi, :], probs[t][lo:hi, :], rsum[t][lo:hi, :]
                    )

                vector.wait_ge(psem, p_ht)
                vector.wait_ge(bqsem, 16)
                nc.vector.tensor_scalar_add(
                    biasc[:, :], ht_ps[:, :NAB], bqh_sb[:, :]
                ).then_inc(vsem, 1)
                vector.wait_ge(psem, p_qt[0])
                nc.vector.tensor_copy(qtt[0][:, :], qt_ps[0][:, :]).then_inc(
                    vsem, 1
                )
                vector.drain()
                for g in range(NCHUNK):
                    if g == 2:
                        for b in range(1, B):
                            vector.wait_ge(psem, p_qt[b])
                            nc.vector.tensor_copy(
                                qtt[b][:, :], qt_ps[b % 2][:, :]
                            ).then_inc(vsem, 1)
                    b = g // CPB
                    for pt in range(_parts_of(g)):
                        if g >= 2:
                            vector.wait_ge(psem, p_last(g - 2))
                        for k in _part_ks(g, pt):
                            ci = b * A_PER + (g % CPB) * CH + k
                            ins = nc.vector.tensor_scalar_add(
                                tin[g % 2][:, k * L_Q:(k + 1) * L_Q],
                                qtt[b][:, :],
                                biasc[:, ci:ci + 1],
                            )
                        ins.then_inc(vsem, 1)
                    if g == GPT + 1:
                        vector.wait_ge(psem, p_last(GPT - 1))
                        for j in range(2):
                            nc.vector.tensor_copy(
                                sc0[:, :][32 * j:32 * (j + 1), :],
                                banks[j][32 * j:32 * (j + 1), :],
                            )
                    if g == GPT + 2:
                        for j in range(2, 4):
                            nc.vector.tensor_copy(
                                sc0[:, :][32 * j:32 * (j + 1), :],
                                banks[j][32 * j:32 * (j + 1), :],
                            )
                        vector.drain()
                        nc.vector.tensor_reduce(
                            negmax[0][:, :], sc0[:, :],
                            axis=AX.X, op=ALU.max, negate=True,
                        ).then_inc(vsem, 1)
                    if g == NCHUNK - 1:
                        # tile-1 groups 0,1 complete after chunk 5
                        vector.wait_ge(psem, p_last(NCHUNK - 3))
                        for j in range(2):
                            ins = negmax_piece(1, j)
                        ins.then_inc(vsem, 1)
                        vector.wait_ge(asem, a_exp0)
                        scale_rows(0, 0, 128).then_inc(vsem, 1)
                vector.wait_ge(psem, p_last(NCHUNK - 1))
                for j in range(2, 4):
                    ins = negmax_piece(1, j)
                ins.then_inc(vsem, 1)
                vector.wait_ge(asem, a_exp1a)
                scale_rows(1, 0, 64).then_inc(vsem, 1)
                vector.wait_ge(asem, a_exp1b)
                scale_rows(1, 64, 128).then_inc(vsem, 1)

    return nc


def _get_program():
    if "nc" not in _CACHE:
        _CACHE["nc"] = build_program()
    return _CACHE["nc"]


def _pmajor(a, nchunks):
    """(nchunks*128, X) -> (128, nchunks, X) partition-major layout."""
    x = a.reshape(nchunks, 128, a.shape[-1])
    return np.ascontiguousarray(x.transpose(1, 0, 2))


def _make_in_maps(inputs):
    import ml_dtypes

    query = np.asarray(inputs["query"], dtype=np.float32)
    decoder_states = np.asarray(inputs["decoder_states"], dtype=np.float32)
    Wq = np.asarray(inputs["Wq"], dtype=np.float32)
    Wh = np.asarray(inputs["Wh"], dtype=np.float32)
    wqh = _pmajor(np.vstack([Wq, Wh]), NWC)
    w2v = np.asarray(inputs["w2"], np.float32).reshape(H)
    w2oh = np.zeros((H, 32, 32), dtype=np.float32)
    w2oh[:, np.arange(32), np.arange(32)] = w2v[:, None]
    w2oh = w2oh.astype(ml_dtypes.bfloat16)
    bqh = np.ascontiguousarray(
        (np.asarray(inputs["bq"], np.float32)
         + np.asarray(inputs["bh"], np.float32)).reshape(H, 1)
    )
    qTf = query.transpose(1, 2, 0)  # (B, Q, L_q)
    qT = np.stack([_pmajor(qTf[b], NQC) for b in range(B)])
    in_maps = []
    for c in range(N_CORES):
        dslice = decoder_states[c * A_PER:(c + 1) * A_PER]
        # (D, B*A): column (b*A + a) holds decoder_states[a, b, :]
        dT = _pmajor(
            dslice.transpose(2, 1, 0).reshape(D_SIZE, NAB), NDC
        )
        in_maps.append({
            "qT": qT,
            "dT": dT,
            "wqh": wqh,
            "w2oh": w2oh,
            "bqh": bqh,
        })
    return in_maps


def kernel(query, decoder_states, query_mask, Wq, bq, Wh, bh, w2, b2):
    from concourse.bass_utils import run_bass_kernel_spmd

    mask = np.asarray(query_mask)
    nc = _get_program()
    in_maps = _make_in_maps({
        "query": query, "decoder_states": decoder_states,
        "Wq": Wq, "Wh": Wh, "w2": w2, "bq": bq, "bh": bh,
    })
    res = run_bass_kernel_spmd(nc, in_maps, list(range(N_CORES))).results
    perm = _row_perm()  # (A_PER, B) -> raw row
    out = np.empty((L_A, B, L_Q), dtype=np.float32)
    for c in range(N_CORES):
        out[c * A_PER:(c + 1) * A_PER] = res[c]["raw"][perm, :]

    if not mask.all():
        # exact post-exp masking + renormalization, host-side
        m = mask.T.astype(np.float32)  # (B, L_q)
        out = out * m[None, :, :]
        out = out / out.sum(axis=-1, keepdims=True)
    return out
